# revision 7
# baseline (speedup 1.0000x reference)
"""AdaptiveSpanAttention Trainium2 kernel (8 NeuronCores).

Sharding: core c -> (batch b = c//2, head-group g = c%2).
Each core computes, for its batch and its 8 heads:
  Q/K/V projections, anti-causal (j>=i) attention with adaptive-span
  mask, renormalization, and a partial output projection
  y_part = Out_g @ Wo[:, e_slice].T  (contraction over its 512 channels).
Host combines: y[b] = y_part[2b] + y_part[2b+1] + bo.

All matmuls in bf16 (f32 PSUM accumulation). Span-mask ramp in fp16
(exact for integer distances). No collectives.
"""
import sys

sys.path.insert(0, "/opt/trn_rl_repo")

from contextlib import ExitStack

import ml_dtypes
import numpy as np

import concourse.bass as bass
import concourse.tile as tile
from concourse import bacc, mybir
from concourse.bass_utils import run_bass_kernel_spmd

BF16 = mybir.dt.bfloat16
F16 = mybir.dt.float16
F32 = mybir.dt.float32

B, T, D, H = 4, 1024, 1024, 16
DH = 64          # head dim
R = 256.0
HC = 8           # heads per core
E = 512          # channels per core (HC * DH)
N_CORES = 8
TCH = 512        # t-chunk width (PSUM f32 free-dim limit)
NT = T // TCH    # 2 t-chunks
ST = T // 128    # 8 s-tiles
DT = D // 128    # 8 d-tiles

_NC_CACHE = {}


def causal_width(st, tch):
    """Valid query-column width of block (s_tile=st, t_chunk=tch).

    Block covers s in [128*st, 128*st+128), t in [512*tch, 512*tch+512).
    Valid cells need s >= t, i.e. t' < delta + 128 with
    delta = 128*st - 512*tch. Width 0 means the block is entirely invalid.
    """
    delta = 128 * st - 512 * tch
    return max(0, min(TCH, delta + 128))


def build_nc():
    if "nc" in _NC_CACHE:
        return _NC_CACHE["nc"]
    nc = bacc.Bacc("TRN2", target_bir_lowering=False, debug=False, num_devices=1)

    # ---- DRAM parameters (per-core shards prepared on host) ----
    xT_d = nc.declare_dram_parameter("xT", [D, T], BF16, isOutput=False)
    WqT_d = nc.declare_dram_parameter("WqT", [D, E], BF16, isOutput=False)
    WkT_d = nc.declare_dram_parameter("WkT", [D, E], BF16, isOutput=False)
    WvT_d = nc.declare_dram_parameter("WvT", [D, E], BF16, isOutput=False)
    WoT_d = nc.declare_dram_parameter("WoT", [E, D], BF16, isOutput=False)
    WspT_d = nc.declare_dram_parameter("WspT", [D, HC], BF16, isOutput=False)
    bspan_d = nc.declare_dram_parameter("bspan", [1, HC], F32, isOutput=False)
    # Cneg[k, s', t'] = -(128k + s' - t')/R, or -60000 where 128k+s'-t' < 0
    cneg_d = nc.declare_dram_parameter("cneg", [ST, 128, TCH], F16, isOutput=False)
    yp_d = nc.declare_dram_parameter("yp", [T, D], F32, isOutput=True)

    with tile.TileContext(nc) as tc, ExitStack() as ctx:
        # ---------------- pools ----------------
        consts = ctx.enter_context(tc.tile_pool(name="consts", bufs=1))
        xp = ctx.enter_context(tc.tile_pool(name="xp", bufs=1))
        wp = ctx.enter_context(tc.tile_pool(name="wp", bufs=1))
        qkp = ctx.enter_context(tc.tile_pool(name="qkp", bufs=1))
        vp = ctx.enter_context(tc.tile_pool(name="vp", bufs=1))
        outp = ctx.enter_context(tc.tile_pool(name="outp", bufs=1))
        scr = ctx.enter_context(tc.tile_pool(name="scr", bufs=3))
        ysb = ctx.enter_context(tc.tile_pool(name="ysb", bufs=3))

        ps_tiny = ctx.enter_context(tc.tile_pool(name="ps_tiny", bufs=1, space="PSUM"))
        ps_proj = ctx.enter_context(tc.tile_pool(name="ps_proj", bufs=2, space="PSUM"))
        ps_sc = ctx.enter_context(tc.tile_pool(name="ps_sc", bufs=2, space="PSUM"))
        ps_bc = ctx.enter_context(tc.tile_pool(name="ps_bc", bufs=1, space="PSUM"))
        ps_out = ctx.enter_context(tc.tile_pool(name="ps_out", bufs=2, space="PSUM"))

        # ---------------- constants / inputs to SBUF ----------------
        ones_row = consts.tile([1, 128], BF16)
        nc.vector.memset(ones_row[:], 1.0)

        cneg_sb = []
        for k in range(ST):
            ct = consts.tile([128, TCH], F16, tag="cneg", bufs=ST)
            nc.sync.dma_start(ct[:], cneg_d[k])
            cneg_sb.append(ct)
        zero_sb = consts.tile([128, TCH], F16)
        nc.vector.memset(zero_sb[:], 0.0)

        bspan_sb = consts.tile([1, HC], F32)
        nc.sync.dma_start(bspan_sb[:], bspan_d[:, :])

        xT_sb = []
        for dt_i in range(DT):
            t_ = xp.tile([128, T], BF16, tag="xT", bufs=DT)
            nc.sync.dma_start(t_[:], xT_d[128 * dt_i:128 * (dt_i + 1), :])
            xT_sb.append(t_)

        wq_sb, wk_sb, wv_sb = [], [], []
        for dt_i in range(DT):
            for lst, dram, tag in ((wq_sb, WqT_d, "wq"), (wk_sb, WkT_d, "wk"),
                                   (wv_sb, WvT_d, "wv")):
                t_ = wp.tile([128, E], BF16, tag=tag, bufs=DT)
                nc.sync.dma_start(t_[:], dram[128 * dt_i:128 * (dt_i + 1), :])
                lst.append(t_)

        wsp_sb = []
        for dt_i in range(DT):
            t_ = wp.tile([128, HC], BF16, tag="wsp", bufs=DT)
            nc.sync.dma_start(t_[:], WspT_d[128 * dt_i:128 * (dt_i + 1), :])
            wsp_sb.append(t_)

        wo_sb = []
        for h in range(HC):
            t_ = wp.tile([64, D], BF16, tag="wo", bufs=HC)
            nc.sync.dma_start(t_[:], WoT_d[64 * h:64 * (h + 1), :])
            wo_sb.append(t_)

        # ---------------- span net ----------------
        # mean over t (sum; /T folded into sigmoid scale)
        msum = consts.tile([128, DT], BF16)
        with nc.allow_low_precision(reason="span-net mean in bf16 is plenty"):
            for dt_i in range(DT):
                nc.vector.tensor_reduce(
                    msum[:, dt_i:dt_i + 1], xT_sb[dt_i][:], mybir.AxisListType.X,
                    mybir.AluOpType.add)
        zlog = ps_tiny.tile([1, HC], F32, tag="tiny")
        for dt_i in range(DT):
            nc.tensor.matmul(zlog[:], msum[:, dt_i:dt_i + 1], wsp_sb[dt_i][:],
                             start=(dt_i == 0), stop=(dt_i == DT - 1))
        # logit = zlog/(T*T) ... careful: zlog = sum_d sum_t x * Wspan
        # mean = sum_t x / T, so logit = zlog / T + bspan
        zrow = consts.tile([1, HC], F32)
        nc.vector.scalar_tensor_tensor(
            zrow[:], zlog[:], 1.0 / T, bspan_sb[:],
            op0=mybir.AluOpType.mult, op1=mybir.AluOpType.add)
        sig = consts.tile([1, HC], BF16)
        nc.scalar.activation(sig[:], zrow[:], mybir.ActivationFunctionType.Sigmoid)
        # a = 1 + (T/R) * sigmoid  (broadcast down 128 partitions via PE)
        a_ps = ps_tiny.tile([128, HC], F32, tag="tiny")
        nc.tensor.matmul(a_ps[:], ones_row[:], sig[:], start=True, stop=True)
        a_sb = consts.tile([128, HC], F32)
        nc.scalar.activation(a_sb[:], a_ps[:], mybir.ActivationFunctionType.Identity,
                             scale=T / R, bias=1.0)

        # ---------------- Q/K projections (transposed layout) ----------------
        # QT[e, t] = sum_d WqT[d, e] * xT[d, t]
        qt_sb = [qkp.tile([128, T], BF16, tag="qt", name=f"qt{i}", bufs=4) for i in range(4)]
        kt_sb = [qkp.tile([128, T], BF16, tag="kt", name=f"kt{i}", bufs=4) for i in range(4)]
        for dst, w_sb in ((qt_sb, wq_sb), (kt_sb, wk_sb)):
            for et in range(4):
                for tch in range(NT):
                    ps = ps_proj.tile([128, TCH], F32, tag="proj")
                    for dt_i in range(DT):
                        nc.tensor.matmul(
                            ps[:],
                            w_sb[dt_i][:, 128 * et:128 * (et + 1)],
                            xT_sb[dt_i][:, TCH * tch:TCH * (tch + 1)],
                            start=(dt_i == 0), stop=(dt_i == DT - 1))
                    nc.vector.tensor_copy(
                        dst[et][:, TCH * tch:TCH * (tch + 1)], ps[:])

        # ---------------- V (natural layout, augmented with ones col) -------
        # V[t, e] = sum_d xT[d, t] * WvT[d, e]; stored per s-tile as
        # v_aug[st][p, h, 0:64] = V[128*st+p, 64h+j], v_aug[..., 64] = 1.0
        v_aug = []
        for st in range(ST):
            va = vp.tile([128, HC, DH + 1], BF16, tag="vaug", bufs=ST)
            nc.vector.memset(va[:], 1.0)
            ps = ps_proj.tile([128, E], F32, tag="proj")
            for dt_i in range(DT):
                nc.tensor.matmul(
                    ps[:],
                    xT_sb[dt_i][:, 128 * st:128 * (st + 1)],
                    wv_sb[dt_i][:],
                    start=(dt_i == 0), stop=(dt_i == DT - 1))
            nc.vector.tensor_copy(
                va[:, :, 0:DH], ps[:].rearrange("p (h d) -> p h d", h=HC))
            v_aug.append(va)

        # ---------------- attention ----------------
        # per (head, t-chunk): scoresT -> exp -> span/causal mask -> attn@V
        out_t = [[None] * NT for _ in range(HC)]
        for tch in range(NT):
            for h in range(HC):
                et, hp = h // 2, (h % 2) * 64
                pout = ps_out.tile([DH + 1, TCH], F32, tag="pout")
                first_st = 4 * tch
                for st in range(first_st, ST):
                    w = causal_width(st, tch)
                    k = st - first_st  # Cneg class: delta = 128*k
                    sp = ps_sc.tile([128, TCH], F32, tag="sc")
                    nc.tensor.matmul(
                        sp[:, 0:w],
                        kt_sb[et][hp:hp + DH, 128 * st:128 * (st + 1)],
                        qt_sb[et][hp:hp + DH, TCH * tch:TCH * tch + w],
                        start=True, stop=True)
                    p_sb = scr.tile([128, TCH], BF16, tag="p")
                    nc.scalar.activation(
                        p_sb[:, 0:w], sp[:, 0:w],
                        mybir.ActivationFunctionType.Exp, scale=1.0 / 8.0)
                    # span + causal mask: pm = min(max(a_h + cneg, 0), 1) * p
                    mt = scr.tile([128, TCH], F16, tag="mt")
                    nc.vector.scalar_tensor_tensor(
                        mt[:, 0:w], cneg_sb[k][:, 0:w], a_sb[:, h:h + 1],
                        zero_sb[:, 0:w],
                        op0=mybir.AluOpType.add, op1=mybir.AluOpType.max)
                    nc.vector.scalar_tensor_tensor(
                        p_sb[:, 0:w], mt[:, 0:w], 1.0, p_sb[:, 0:w],
                        op0=mybir.AluOpType.min, op1=mybir.AluOpType.mult)
                    nc.tensor.matmul(
                        pout[:, 0:w], v_aug[st][:, h, :], p_sb[:, 0:w],
                        start=(st == first_st), stop=(st == ST - 1),
                        skip_group_check=True)
                # normalize: rows 0:64 are numerator, row 64 is denominator W
                rw = scr.tile([1, TCH], BF16, tag="rw")
                with nc.allow_low_precision(reason="denominator recip in bf16"):
                    nc.vector.reciprocal(rw[:], pout[DH:DH + 1, :])
                bc = ps_bc.tile([DH, TCH], F32, tag="bc")
                nc.tensor.matmul(bc[:], ones_row[:, 0:DH], rw[:],
                                 start=True, stop=True)
                bc_sb = scr.tile([DH, TCH], BF16, tag="bcsb")
                nc.vector.tensor_copy(bc_sb[:], bc[:])
                ot = outp.tile([DH, TCH], BF16, tag="out", bufs=2 * HC)
                nc.vector.scalar_tensor_tensor(
                    ot[:], pout[0:DH, :], 1.0, bc_sb[:],
                    op0=mybir.AluOpType.mult, op1=mybir.AluOpType.mult)
                out_t[h][tch] = ot

            # ---------------- output projection for this chunk ----------------
            for tt in range(4 * tch, 4 * (tch + 1)):
                toff = 128 * tt - TCH * tch
                for nch in range(NT):
                    yps = ps_proj.tile([128, TCH], F32, tag="proj")
                    for h in range(HC):
                        nc.tensor.matmul(
                            yps[:],
                            out_t[h][tch][:, toff:toff + 128],
                            wo_sb[h][:, TCH * nch:TCH * (nch + 1)],
                            start=(h == 0), stop=(h == HC - 1))
                    yo = ysb.tile([128, TCH], F32, tag="y")
                    nc.vector.tensor_copy(yo[:], yps[:])
                    nc.sync.dma_start(
                        yp_d[128 * tt:128 * (tt + 1), TCH * nch:TCH * (nch + 1)],
                        yo[:])

    nc.compile()
    _NC_CACHE["nc"] = nc
    return nc


def _prep_core_inputs(x, Wq, Wk, Wv, Wo, Wspan, bspan, cneg):
    bf = ml_dtypes.bfloat16
    in_maps = []
    for c in range(N_CORES):
        b, g = c // 2, c % 2
        hs = slice(E * g, E * (g + 1))
        in_maps.append({
            "xT": np.ascontiguousarray(x[b].T).astype(bf),
            "WqT": np.ascontiguousarray(Wq[hs, :].T).astype(bf),
            "WkT": np.ascontiguousarray(Wk[hs, :].T).astype(bf),
            "WvT": np.ascontiguousarray(Wv[hs, :].T).astype(bf),
            "WoT": np.ascontiguousarray(Wo[:, hs].T).astype(bf),
            "WspT": np.ascontiguousarray(Wspan[HC * g:HC * (g + 1), :].T).astype(bf),
            "bspan": np.asarray(bspan[HC * g:HC * (g + 1)], np.float32).reshape(1, HC),
            "cneg": cneg,
        })
    return in_maps


def _make_cneg():
    sp = np.arange(128, dtype=np.float32)[:, None]
    tp = np.arange(TCH, dtype=np.float32)[None, :]
    tiles = []
    for k in range(ST):
        d = 128.0 * k + sp - tp
        ramp = -d / R
        ramp = np.where(d < 0, -60000.0, ramp)
        tiles.append(ramp)
    return np.stack(tiles).astype(np.float16)


def kernel(x, Wq, Wk, Wv, Wo, bo, Wspan, bspan):
    x = np.asarray(x, np.float32)
    Wq = np.asarray(Wq, np.float32)
    Wk = np.asarray(Wk, np.float32)
    Wv = np.asarray(Wv, np.float32)
    Wo = np.asarray(Wo, np.float32)
    bo = np.asarray(bo, np.float32)
    Wspan = np.asarray(Wspan, np.float32)
    bspan = np.asarray(bspan, np.float32)

    nc = build_nc()
    in_maps = _prep_core_inputs(x, Wq, Wk, Wv, Wo, Wspan, bspan, _make_cneg())
    res = run_bass_kernel_spmd(nc, in_maps, core_ids=list(range(N_CORES)))
    y = np.empty((B, T, D), np.float32)
    for b in range(B):
        y[b] = res.results[2 * b]["yp"] + res.results[2 * b + 1]["yp"] + bo
    return y


# revision 22
# speedup vs baseline: 22171.2553x; 22171.2553x over previous
"""AdaptiveSpanAttention Trainium2 kernel (8 NeuronCores).

Sharding: core c -> (batch b = c//2, head-group g = c%2).
Each core computes, for its batch and its 8 heads:
  Q/K/V projections, anti-causal (j>=i) attention with adaptive-span
  mask, renormalization, and a partial output projection
  y_part = Out_g @ Wo[:, e_slice].T  (contraction over its 512 channels).
Host combines: y[b] = y_part[2b] + y_part[2b+1] + bo.

All matmuls in bf16 (f32 PSUM accumulation). Span-mask ramp in fp16
(exact for integer distances). No collectives.
"""
import sys

sys.path.insert(0, "/opt/trn_rl_repo")

from contextlib import ExitStack

import ml_dtypes
import numpy as np

import concourse.bass as bass
import concourse.tile as tile
from concourse import bacc, mybir
from concourse.bass_utils import run_bass_kernel_spmd

BF16 = mybir.dt.bfloat16
F16 = mybir.dt.float16
F32 = mybir.dt.float32

B, T, D, H = 4, 1024, 1024, 16
DH = 64          # head dim
R = 256.0
HC = 8           # heads per core
E = 512          # channels per core (HC * DH)
N_CORES = 8
TCH = 512        # t-chunk width (PSUM f32 free-dim limit)
NT = T // TCH    # 2 t-chunks
ST = T // 128    # 8 s-tiles
DT = D // 128    # 8 d-tiles

_NC_CACHE = {}

Z_MIN = 416.0  # verified on host per-call; span-mask restriction exact when z >= Z_MIN


def causal_width(st, tch):
    """Valid query-column width of block (s_tile=st, t_chunk=tch).

    Block covers s in [128*st, 128*st+128), t in [512*tch, 512*tch+512).
    Valid cells need s >= t, i.e. t' < delta + 128 with
    delta = 128*st - 512*tch.
    """
    delta = 128 * st - 512 * tch
    return max(0, min(TCH, delta + 128))


def span_width(st, tch, span_full):
    """Columns [0, m_w) where the span mask can differ from 1 (given z >= Z_MIN)."""
    delta = 128 * st - 512 * tch
    w = causal_width(st, tch)
    if span_full:
        return w
    return max(0, min(w, delta + 127 - int(Z_MIN)))


def build_nc(span_full=False):
    key = ("nc", span_full)
    if key in _NC_CACHE:
        return _NC_CACHE[key]
    nc = bacc.Bacc("TRN2", target_bir_lowering=False, debug=False, num_devices=1)

    # ---- DRAM parameters (per-core shards prepared on host) ----
    xT_d = nc.declare_dram_parameter("xT", [D, T], BF16, isOutput=False)
    WqT_d = nc.declare_dram_parameter("WqT", [D, E], BF16, isOutput=False)
    WkT_d = nc.declare_dram_parameter("WkT", [D, E], BF16, isOutput=False)
    WvT_d = nc.declare_dram_parameter("WvT", [D, E], BF16, isOutput=False)
    WoT_d = nc.declare_dram_parameter("WoT", [E, D], BF16, isOutput=False)
    WspT_d = nc.declare_dram_parameter("WspT", [D, HC], BF16, isOutput=False)
    bspan_d = nc.declare_dram_parameter("bspan", [1, HC], F32, isOutput=False)
    # cneg[k, s', t'] = -(128k + s' - t')/R, or -60000 where 128k+s'-t' < 0
    cneg_d = nc.declare_dram_parameter("cneg", [ST, 128, TCH], F16, isOutput=False)
    # c01[k, s', j] = 1.0 if s' >= j else 0.0  (causal 0/1 for t' = 128k + j)
    c01_d = nc.declare_dram_parameter("c01", [4, 128, 128], F16, isOutput=False)
    yp_d = nc.declare_dram_parameter("yp", [T, D], F32, isOutput=True)

    with tile.TileContext(nc) as tc, ExitStack() as ctx:
        # ---------------- pools ----------------
        consts = ctx.enter_context(tc.tile_pool(name="consts", bufs=1))
        xp = ctx.enter_context(tc.tile_pool(name="xp", bufs=1))
        wp = ctx.enter_context(tc.tile_pool(name="wp", bufs=1))
        qkp = ctx.enter_context(tc.tile_pool(name="qkp", bufs=1))
        vp = ctx.enter_context(tc.tile_pool(name="vp", bufs=1))
        outp = ctx.enter_context(tc.tile_pool(name="outp", bufs=1))
        scr = ctx.enter_context(tc.tile_pool(name="scr", bufs=3))
        ysb = ctx.enter_context(tc.tile_pool(name="ysb", bufs=3))

        ps_proj = ctx.enter_context(tc.tile_pool(name="ps_proj", bufs=2, space="PSUM"))
        ps_sc = ctx.enter_context(tc.tile_pool(name="ps_sc", bufs=2, space="PSUM"))
        ps_out = ctx.enter_context(tc.tile_pool(name="ps_out", bufs=4, space="PSUM"))

        # ---------------- critical-path loads: xT, Wq, Wk ----------------
        ones_row = consts.tile([1, 128], BF16)
        nc.vector.memset(ones_row[:], 1.0)

        xT_sb, wq_sb, wk_sb = [], [], []
        for dt_i in range(DT):
            t_ = xp.tile([128, T], BF16, tag="xT", bufs=DT, name=f"xT{dt_i}")
            nc.sync.dma_start(t_[:], xT_d[128 * dt_i:128 * (dt_i + 1), :])
            xT_sb.append(t_)
            for lst, dram, tag in ((wq_sb, WqT_d, "wq"), (wk_sb, WkT_d, "wk")):
                t_ = wp.tile([128, E], BF16, tag=tag, bufs=DT,
                             name=f"{tag}{dt_i}")
                nc.sync.dma_start(t_[:], dram[128 * dt_i:128 * (dt_i + 1), :])
                lst.append(t_)

        # ---------------- Q/K projections (transposed layout) ----------------
        # QT[e, t] = sum_d WqT[d, e] * xT[d, t]; psum -> bf16 copies on ACT
        qt_sb = [qkp.tile([128, T], BF16, tag="qt", name=f"qt{i}", bufs=4)
                 for i in range(4)]
        kt_sb = [qkp.tile([128, T], BF16, tag="kt", name=f"kt{i}", bufs=4)
                 for i in range(4)]

        def emit_qtkt(et_list, copy_eng="act"):
            for dst, w_sb in ((qt_sb, wq_sb), (kt_sb, wk_sb)):
                for et in et_list:
                    for tch in range(NT):
                        ps = ps_proj.tile([128, TCH], F32, tag="proj")
                        for dt_i in range(DT):
                            nc.tensor.matmul(
                                ps[:],
                                w_sb[dt_i][:, 128 * et:128 * (et + 1)],
                                xT_sb[dt_i][:, TCH * tch:TCH * (tch + 1)],
                                start=(dt_i == 0), stop=(dt_i == DT - 1))
                        if copy_eng == "act":
                            nc.scalar.copy(
                                dst[et][:, TCH * tch:TCH * (tch + 1)], ps[:])
                        else:
                            nc.vector.tensor_copy(
                                dst[et][:, TCH * tch:TCH * (tch + 1)], ps[:])

        emit_qtkt([0, 1])

        # ---------------- V (natural layout, ones-augmented) ----------------
        # v_aug[st][p, h, 0:64] = V[128*st+p, 64h+j]; v_aug[st][p, h, 64:128] = 1
        # (64 ones columns make attn@V produce the denominator W broadcast
        #  across psum partitions 64:128)
        wv_sb = []
        for dt_i in range(DT):
            t_ = wp.tile([128, E], BF16, tag="wv", bufs=DT, name=f"wv{dt_i}")
            nc.sync.dma_start(t_[:], WvT_d[128 * dt_i:128 * (dt_i + 1), :])
            wv_sb.append(t_)

        v_aug = [None] * ST

        def emit_v(st):
            va = vp.tile([128, HC, 2 * DH], BF16, tag="vaug", bufs=ST,
                         name=f"vaug{st}")
            nc.gpsimd.memset(va[:, :, DH:2 * DH], 1.0)
            ps = ps_proj.tile([128, E], F32, tag="proj")
            for dt_i in range(DT):
                nc.tensor.matmul(
                    ps[:],
                    xT_sb[dt_i][:, 128 * st:128 * (st + 1)],
                    wv_sb[dt_i][:],
                    start=(dt_i == 0), stop=(dt_i == DT - 1))
            nc.vector.tensor_copy(
                va[:, :, 0:DH], ps[:].rearrange("p (h d) -> p h d", h=HC))
            v_aug[st] = va

        emit_v(0)
        emit_v(1)

        # ---------------- remaining loads ----------------
        bspan_sb = consts.tile([1, HC], F32)
        nc.sync.dma_start(bspan_sb[:], bspan_d[:, :])
        wsp_sb = []
        for dt_i in range(DT):
            t_ = wp.tile([128, HC], BF16, tag="wsp", bufs=DT, name=f"wsp{dt_i}")
            nc.sync.dma_start(t_[:], WspT_d[128 * dt_i:128 * (dt_i + 1), :])
            wsp_sb.append(t_)
        wo_sb = []
        for j in range(4):
            t_ = wp.tile([128, D], BF16, tag="wo", bufs=4, name=f"wo{j}")
            nc.sync.dma_start(t_[:], WoT_d[128 * j:128 * (j + 1), :])
            wo_sb.append(t_)
        cneg_sb = []
        for k in range(ST):
            ct = consts.tile([128, TCH], F16, tag="cneg", bufs=ST,
                             name=f"cneg{k}")
            nc.sync.dma_start(ct[:], cneg_d[k])
            cneg_sb.append(ct)
        c01_sb = []
        for k in range(4):
            ct2 = consts.tile([128, 128], F16, tag="c01", bufs=4,
                              name=f"c01_{k}")
            nc.sync.dma_start(ct2[:], c01_d[k])
            c01_sb.append(ct2)

        # ---------------- span net ----------------
        # logit = (sum_t x)/T @ WspanT + bspan; a = 1 + (T/R)*sigmoid(logit)
        msum = consts.tile([128, DT], BF16)
        with nc.allow_low_precision(reason="span-net mean in bf16 is plenty"):
            for dt_i in range(DT):
                nc.vector.tensor_reduce(
                    msum[:, dt_i:dt_i + 1], xT_sb[dt_i][:],
                    mybir.AxisListType.X, mybir.AluOpType.add)
        zlog = ps_sc.tile([1, HC], F32, tag="sc", padded_shape=[128, TCH])
        for dt_i in range(DT):
            nc.tensor.matmul(zlog[:], msum[:, dt_i:dt_i + 1], wsp_sb[dt_i][:],
                             start=(dt_i == 0), stop=(dt_i == DT - 1))
        zrow = consts.tile([1, HC], F32)
        nc.vector.scalar_tensor_tensor(
            zrow[:], zlog[:], 1.0 / T, bspan_sb[:],
            op0=mybir.AluOpType.mult, op1=mybir.AluOpType.add)
        sig = consts.tile([1, HC], BF16)
        nc.scalar.activation(sig[:], zrow[:],
                             mybir.ActivationFunctionType.Sigmoid)
        a_ps = ps_sc.tile([128, HC], F32, tag="sc", padded_shape=[128, TCH])
        nc.tensor.matmul(a_ps[:], ones_row[:], sig[:], start=True, stop=True)
        a_sb = consts.tile([128, HC], F32)
        nc.scalar.activation(a_sb[:], a_ps[:],
                             mybir.ActivationFunctionType.Identity,
                             scale=T / R, bias=1.0)

        # ---------------- attention ----------------
        # out_pair[j][tch] holds heads 2j (parts 0:64) and 2j+1 (parts 64:128)
        out_pair = [[outp.tile([128, TCH], BF16, tag="out", bufs=8,
                               name=f"op{j}_{c}") for c in range(NT)]
                    for j in range(4)]

        def attn_group(tch, hg, v_prefetch=False):
            first_st = 4 * tch
            heads = [4 * hg + i for i in range(4)]
            pouts = {}
            for h in heads:
                pouts[h] = ps_out.tile([128, TCH], F32, tag="pout",
                                       name=f"pout{h}_{tch}")
            for st in range(first_st, ST):
                if v_prefetch and st + 2 < ST and v_aug[st + 2] is None:
                    emit_v(st + 2)
                w = causal_width(st, tch)
                m_w = span_width(st, tch, span_full)
                k = st - first_st  # delta = 128*k
                sps, pbs = {}, {}
                for h in heads:
                    et, hp = h // 2, (h % 2) * 64
                    sp = ps_sc.tile([128, TCH], F32, tag="sc",
                                    name=f"sp{h}_{st}")
                    nc.tensor.matmul(
                        sp[:, 0:w],
                        kt_sb[et][hp:hp + DH, 128 * st:128 * (st + 1)],
                        qt_sb[et][hp:hp + DH, TCH * tch:TCH * tch + w],
                        start=True, stop=True)
                    sps[h] = sp
                for h in heads:
                    p_sb = scr.tile([128, TCH], BF16, tag="p", bufs=10,
                                    name=f"p{h}_{st}")
                    nc.scalar.activation(
                        p_sb[:, 0:w], sps[h][:, 0:w],
                        mybir.ActivationFunctionType.Exp, scale=1.0 / 8.0)
                    pbs[h] = p_sb
                for h in heads:
                    p_sb = pbs[h]
                    if k <= 3:
                        # diagonal block: causal zeroing on t' in [128k, w)
                        d0 = 128 * k
                        ceng = nc.vector if tch == 0 else nc.gpsimd
                        ceng.tensor_mul(
                            p_sb[:, d0:w], p_sb[:, d0:w],
                            c01_sb[k][:, 0:w - d0])
                    if m_w > 0:
                        # span mask: pm = min(max(a_h + cneg, 0), 1) * p
                        mt = scr.tile([128, TCH], F16, tag="mt", bufs=6,
                                      name=f"mt{h}_{st}")
                        nc.gpsimd.tensor_scalar(
                            mt[:, 0:m_w], cneg_sb[k][:, 0:m_w],
                            a_sb[:, h:h + 1], 0.0,
                            op0=mybir.AluOpType.add, op1=mybir.AluOpType.max)
                        nc.vector.scalar_tensor_tensor(
                            p_sb[:, 0:m_w], mt[:, 0:m_w], 1.0, p_sb[:, 0:m_w],
                            op0=mybir.AluOpType.min, op1=mybir.AluOpType.mult)
                for h in heads:
                    nc.tensor.matmul(
                        pouts[h][:, 0:w], v_aug[st][:, h, :], pbs[h][:, 0:w],
                        start=(st == first_st), stop=(st == ST - 1),
                        skip_group_check=True)
            for h in heads:
                # rows 0:64 numerator; rows 64:128 denominator W (broadcast)
                et, hp = h // 2, (h % 2) * 64
                pout = pouts[h]
                rw = scr.tile([DH, TCH], F32, tag="rw", bufs=4,
                              name=f"rw{h}")
                with nc.allow_low_precision(reason="denom recip bf16"):
                    nc.vector.reciprocal(rw[:], pout[DH:2 * DH, :])
                nc.vector.scalar_tensor_tensor(
                    out_pair[et][tch][hp:hp + DH, :], pout[0:DH, :], 1.0,
                    rw[:],
                    op0=mybir.AluOpType.mult, op1=mybir.AluOpType.mult)

        def out_proj(tch):
            for tt in range(4 * tch, 4 * (tch + 1)):
                toff = 128 * tt - TCH * tch
                for nch in range(NT):
                    yps = ps_proj.tile([128, TCH], F32, tag="proj")
                    for j in range(4):
                        nc.tensor.matmul(
                            yps[:],
                            out_pair[j][tch][:, toff:toff + 128],
                            wo_sb[j][:, TCH * nch:TCH * (nch + 1)],
                            start=(j == 0), stop=(j == 3))
                    yo = ysb.tile([128, TCH], F32, tag="y")
                    nc.scalar.copy(yo[:], yps[:])
                    nc.sync.dma_start(
                        yp_d[128 * tt:128 * (tt + 1),
                             TCH * nch:TCH * (nch + 1)],
                        yo[:])

        attn_group(0, 0, v_prefetch=True)
        emit_qtkt([2, 3])  # heads 4..7; fills attn bubbles
        attn_group(0, 1)
        out_proj(0)
        attn_group(1, 0)
        attn_group(1, 1)
        out_proj(1)

    nc.compile()
    _NC_CACHE[key] = nc
    return nc


def _prep_core_inputs(x, Wq, Wk, Wv, Wo, Wspan, bspan, cneg, c01):
    bf = ml_dtypes.bfloat16
    in_maps = []
    for c in range(N_CORES):
        b, g = c // 2, c % 2
        hs = slice(E * g, E * (g + 1))
        in_maps.append({
            "c01": c01,
            "xT": np.ascontiguousarray(x[b].T).astype(bf),
            "WqT": np.ascontiguousarray(Wq[hs, :].T).astype(bf),
            "WkT": np.ascontiguousarray(Wk[hs, :].T).astype(bf),
            "WvT": np.ascontiguousarray(Wv[hs, :].T).astype(bf),
            "WoT": np.ascontiguousarray(Wo[:, hs].T).astype(bf),
            "WspT": np.ascontiguousarray(Wspan[HC * g:HC * (g + 1), :].T).astype(bf),
            "bspan": np.asarray(bspan[HC * g:HC * (g + 1)], np.float32).reshape(1, HC),
            "cneg": cneg,
        })
    return in_maps


def _make_c01():
    sp = np.arange(128, dtype=np.float32)[:, None]
    jp = np.arange(128, dtype=np.float32)[None, :]
    return np.stack([(sp - jp >= 0) for _ in range(4)]).astype(np.float16)


def _make_cneg():
    sp = np.arange(128, dtype=np.float32)[:, None]
    tp = np.arange(TCH, dtype=np.float32)[None, :]
    tiles = []
    for k in range(ST):
        d = 128.0 * k + sp - tp
        ramp = -d / R
        ramp = np.where(d < 0, -60000.0, ramp)
        tiles.append(ramp)
    return np.stack(tiles).astype(np.float16)


def kernel(x, Wq, Wk, Wv, Wo, bo, Wspan, bspan):
    x = np.asarray(x, np.float32)
    Wq = np.asarray(Wq, np.float32)
    Wk = np.asarray(Wk, np.float32)
    Wv = np.asarray(Wv, np.float32)
    Wo = np.asarray(Wo, np.float32)
    bo = np.asarray(bo, np.float32)
    Wspan = np.asarray(Wspan, np.float32)
    bspan = np.asarray(bspan, np.float32)

    # span-mask restriction is only exact when every z >= Z_MIN; verify on host
    logits = x.mean(axis=1) @ Wspan.T + bspan
    z = T / (1.0 + np.exp(-logits))
    span_full = bool(z.min() < Z_MIN + 8.0)
    nc = build_nc(span_full=span_full)
    in_maps = _prep_core_inputs(x, Wq, Wk, Wv, Wo, Wspan, bspan, _make_cneg(),
                                _make_c01())
    res = run_bass_kernel_spmd(nc, in_maps, core_ids=list(range(N_CORES)))
    y = np.empty((B, T, D), np.float32)
    for b in range(B):
        y[b] = res.results[2 * b]["yp"] + res.results[2 * b + 1]["yp"] + bo
    return y


# revision 29
# speedup vs baseline: 22521.3852x; 1.0158x over previous
"""AdaptiveSpanAttention Trainium2 kernel (8 NeuronCores).

Sharding: core c -> (batch b = c//2, head-group g = c%2).
Each core computes, for its batch and its 8 heads:
  Q/K/V projections, anti-causal (j>=i) attention with adaptive-span
  mask, renormalization, and a partial output projection
  y_part = Out_g @ Wo[:, e_slice].T  (contraction over its 512 channels).
Host combines: y[b] = y_part[2b] + y_part[2b+1] + bo.

All matmuls in bf16 (f32 PSUM accumulation). Span-mask ramp in fp16
(exact for integer distances). No collectives.
"""
import sys

sys.path.insert(0, "/opt/trn_rl_repo")

from contextlib import ExitStack

import ml_dtypes
import numpy as np

import concourse.bass as bass
import concourse.tile as tile
from concourse import bacc, mybir
from concourse.bass_utils import run_bass_kernel_spmd

BF16 = mybir.dt.bfloat16
F16 = mybir.dt.float16
F32 = mybir.dt.float32

B, T, D, H = 4, 1024, 1024, 16
DH = 64          # head dim
R = 256.0
HC = 8           # heads per core
E = 512          # channels per core (HC * DH)
N_CORES = 8
TCH = 512        # t-chunk width (PSUM f32 free-dim limit)
NT = T // TCH    # 2 t-chunks
ST = T // 128    # 8 s-tiles
DT = D // 128    # 8 d-tiles

_NC_CACHE = {}

Z_MIN = 416.0  # verified on host per-call; span-mask restriction exact when z >= Z_MIN


def causal_width(st, tch):
    """Valid query-column width of block (s_tile=st, t_chunk=tch).

    Block covers s in [128*st, 128*st+128), t in [512*tch, 512*tch+512).
    Valid cells need s >= t, i.e. t' < delta + 128 with
    delta = 128*st - 512*tch.
    """
    delta = 128 * st - 512 * tch
    return max(0, min(TCH, delta + 128))


def span_width(st, tch, span_full):
    """Columns [0, m_w) where the span mask can differ from 1 (given z >= Z_MIN)."""
    delta = 128 * st - 512 * tch
    w = causal_width(st, tch)
    if span_full:
        return w
    return max(0, min(w, delta + 127 - int(Z_MIN)))


def build_nc(span_full=False):
    key = ("nc", span_full)
    if key in _NC_CACHE:
        return _NC_CACHE[key]
    nc = bacc.Bacc("TRN2", target_bir_lowering=False, debug=False, num_devices=1)

    # ---- DRAM parameters (per-core shards prepared on host) ----
    xT_d = nc.declare_dram_parameter("xT", [D, T], BF16, isOutput=False)
    WqT_d = nc.declare_dram_parameter("WqT", [D, E], BF16, isOutput=False)
    WkT_d = nc.declare_dram_parameter("WkT", [D, E], BF16, isOutput=False)
    WvT_d = nc.declare_dram_parameter("WvT", [D, E], BF16, isOutput=False)
    WoT_d = nc.declare_dram_parameter("WoT", [E, D], BF16, isOutput=False)
    WspT_d = nc.declare_dram_parameter("WspT", [D, HC], BF16, isOutput=False)
    bspan_d = nc.declare_dram_parameter("bspan", [1, HC], F32, isOutput=False)
    # cneg[k, s', t'] = -(128k + s' - t')/R, or -60000 where 128k+s'-t' < 0
    cneg_d = nc.declare_dram_parameter("cneg", [ST, 128, TCH], F16, isOutput=False)
    # c01[k, s', j] = 1.0 if s' >= j else 0.0  (causal 0/1 for t' = 128k + j)
    c01_d = nc.declare_dram_parameter("c01", [4, 128, 128], F16, isOutput=False)
    yp_d = nc.declare_dram_parameter("yp", [T, D], F32, isOutput=True)

    with tile.TileContext(nc) as tc, ExitStack() as ctx:
        # ---------------- pools ----------------
        consts = ctx.enter_context(tc.tile_pool(name="consts", bufs=1))
        xp = ctx.enter_context(tc.tile_pool(name="xp", bufs=1))
        wp = ctx.enter_context(tc.tile_pool(name="wp", bufs=1))
        qkp = ctx.enter_context(tc.tile_pool(name="qkp", bufs=1))
        vp = ctx.enter_context(tc.tile_pool(name="vp", bufs=1))
        outp = ctx.enter_context(tc.tile_pool(name="outp", bufs=1))
        scr = ctx.enter_context(tc.tile_pool(name="scr", bufs=3))
        ysb = ctx.enter_context(tc.tile_pool(name="ysb", bufs=3))

        ps_proj = ctx.enter_context(tc.tile_pool(name="ps_proj", bufs=2, space="PSUM"))
        ps_sc = ctx.enter_context(tc.tile_pool(name="ps_sc", bufs=2, space="PSUM"))
        ps_out = ctx.enter_context(tc.tile_pool(name="ps_out", bufs=4, space="PSUM"))

        # ---------------- critical-path loads: xT, Wq, Wk ----------------
        ones_row = consts.tile([1, 128], BF16)
        nc.vector.memset(ones_row[:], 1.0)

        xT_sb, wq_sb, wk_sb = [], [], []
        for dt_i in range(DT):
            t_ = xp.tile([128, T], BF16, tag="xT", bufs=DT, name=f"xT{dt_i}")
            nc.sync.dma_start(t_[:], xT_d[128 * dt_i:128 * (dt_i + 1), :])
            xT_sb.append(t_)
            for lst, dram, tag in ((wq_sb, WqT_d, "wq"), (wk_sb, WkT_d, "wk")):
                t_ = wp.tile([128, E], BF16, tag=tag, bufs=DT,
                             name=f"{tag}{dt_i}")
                nc.sync.dma_start(t_[:], dram[128 * dt_i:128 * (dt_i + 1), :])
                lst.append(t_)

        # ---------------- Q/K projections (transposed layout) ----------------
        # QT[e, t] = sum_d WqT[d, e] * xT[d, t]; psum -> bf16 copies on ACT
        qt_sb = [qkp.tile([128, T], BF16, tag="qt", name=f"qt{i}", bufs=4)
                 for i in range(4)]
        kt_sb = [qkp.tile([128, T], BF16, tag="kt", name=f"kt{i}", bufs=4)
                 for i in range(4)]

        def emit_qtkt(et_list, copy_eng="act"):
            for dst, w_sb in ((qt_sb, wq_sb), (kt_sb, wk_sb)):
                for et in et_list:
                    for tch in range(NT):
                        ps = ps_proj.tile([128, TCH], F32, tag="proj")
                        for dt_i in range(DT):
                            nc.tensor.matmul(
                                ps[:],
                                w_sb[dt_i][:, 128 * et:128 * (et + 1)],
                                xT_sb[dt_i][:, TCH * tch:TCH * (tch + 1)],
                                start=(dt_i == 0), stop=(dt_i == DT - 1))
                        if copy_eng == "act":
                            nc.scalar.copy(
                                dst[et][:, TCH * tch:TCH * (tch + 1)], ps[:])
                        else:
                            nc.vector.tensor_copy(
                                dst[et][:, TCH * tch:TCH * (tch + 1)], ps[:])

        emit_qtkt([0, 1])

        # ---------------- V (natural layout, ones-augmented) ----------------
        # v_aug[st][p, h, 0:64] = V[128*st+p, 64h+j]; v_aug[st][p, h, 64:128] = 1
        # (64 ones columns make attn@V produce the denominator W broadcast
        #  across psum partitions 64:128)
        wv_sb = []
        for dt_i in range(DT):
            t_ = wp.tile([128, E], BF16, tag="wv", bufs=DT, name=f"wv{dt_i}")
            nc.sync.dma_start(t_[:], WvT_d[128 * dt_i:128 * (dt_i + 1), :])
            wv_sb.append(t_)

        v_aug = [None] * ST

        def emit_v(st):
            va = vp.tile([128, HC, 2 * DH], BF16, tag="vaug", bufs=ST,
                         name=f"vaug{st}")
            nc.gpsimd.memset(va[:, :, DH:2 * DH], 1.0)
            ps = ps_proj.tile([128, E], F32, tag="proj")
            for dt_i in range(DT):
                nc.tensor.matmul(
                    ps[:],
                    xT_sb[dt_i][:, 128 * st:128 * (st + 1)],
                    wv_sb[dt_i][:],
                    start=(dt_i == 0), stop=(dt_i == DT - 1))
            nc.vector.tensor_copy(
                va[:, :, 0:DH], ps[:].rearrange("p (h d) -> p h d", h=HC))
            v_aug[st] = va

        emit_v(0)
        emit_v(1)

        # ---------------- remaining loads ----------------
        bspan_sb = consts.tile([1, HC], F32)
        nc.sync.dma_start(bspan_sb[:], bspan_d[:, :])
        wsp_sb = []
        for dt_i in range(DT):
            t_ = wp.tile([128, HC], BF16, tag="wsp", bufs=DT, name=f"wsp{dt_i}")
            nc.sync.dma_start(t_[:], WspT_d[128 * dt_i:128 * (dt_i + 1), :])
            wsp_sb.append(t_)
        wo_sb = []
        for j in range(4):
            t_ = wp.tile([128, D], BF16, tag="wo", bufs=4, name=f"wo{j}")
            nc.sync.dma_start(t_[:], WoT_d[128 * j:128 * (j + 1), :])
            wo_sb.append(t_)
        cneg_sb = []
        for k in range(ST):
            ct = consts.tile([128, TCH], F16, tag="cneg", bufs=ST,
                             name=f"cneg{k}")
            nc.sync.dma_start(ct[:], cneg_d[k])
            cneg_sb.append(ct)
        c01_sb = []
        for k in range(4):
            ct2 = consts.tile([128, 128], F16, tag="c01", bufs=4,
                              name=f"c01_{k}")
            nc.sync.dma_start(ct2[:], c01_d[k])
            c01_sb.append(ct2)

        # ---------------- span net ----------------
        # logit = (sum_t x)/T @ WspanT + bspan; a = 1 + (T/R)*sigmoid(logit)
        msum = consts.tile([128, DT], BF16)
        with nc.allow_low_precision(reason="span-net mean in bf16 is plenty"):
            for dt_i in range(DT):
                nc.vector.tensor_reduce(
                    msum[:, dt_i:dt_i + 1], xT_sb[dt_i][:],
                    mybir.AxisListType.X, mybir.AluOpType.add)
        zlog = ps_sc.tile([1, HC], F32, tag="sc", padded_shape=[128, TCH])
        for dt_i in range(DT):
            nc.tensor.matmul(zlog[:], msum[:, dt_i:dt_i + 1], wsp_sb[dt_i][:],
                             start=(dt_i == 0), stop=(dt_i == DT - 1))
        zrow = consts.tile([1, HC], F32)
        nc.vector.scalar_tensor_tensor(
            zrow[:], zlog[:], 1.0 / T, bspan_sb[:],
            op0=mybir.AluOpType.mult, op1=mybir.AluOpType.add)
        sig = consts.tile([1, HC], BF16)
        nc.scalar.activation(sig[:], zrow[:],
                             mybir.ActivationFunctionType.Sigmoid)
        a_ps = ps_sc.tile([128, HC], F32, tag="sc", padded_shape=[128, TCH])
        nc.tensor.matmul(a_ps[:], ones_row[:], sig[:], start=True, stop=True)
        a_sb = consts.tile([128, HC], F32)
        nc.scalar.activation(a_sb[:], a_ps[:],
                             mybir.ActivationFunctionType.Identity,
                             scale=T / R, bias=1.0)

        # ---------------- attention ----------------
        # out_pair[j][tch] holds heads 2j (parts 0:64) and 2j+1 (parts 64:128)
        out_pair = [[outp.tile([128, TCH], BF16, tag="out", bufs=8,
                               name=f"op{j}_{c}") for c in range(NT)]
                    for j in range(4)]

        def attn_group(tch, hg, v_prefetch=False):
            first_st = 4 * tch
            heads = [4 * hg + i for i in range(4)]
            pouts = {}
            for h in heads:
                pouts[h] = ps_out.tile([128, TCH], F32, tag="pout",
                                       name=f"pout{h}_{tch}")
            for st in range(first_st, ST):
                if v_prefetch and st + 2 < ST and v_aug[st + 2] is None:
                    emit_v(st + 2)
                w = causal_width(st, tch)
                m_w = span_width(st, tch, span_full)
                k = st - first_st  # delta = 128*k
                sps, pbs = {}, {}
                for h in heads:
                    et, hp = h // 2, (h % 2) * 64
                    sp = ps_sc.tile([128, TCH], F32, tag="sc",
                                    name=f"sp{h}_{st}")
                    nc.tensor.matmul(
                        sp[:, 0:w],
                        kt_sb[et][hp:hp + DH, 128 * st:128 * (st + 1)],
                        qt_sb[et][hp:hp + DH, TCH * tch:TCH * tch + w],
                        start=True, stop=True)
                    sps[h] = sp
                for h in heads:
                    p_sb = scr.tile([128, TCH], BF16, tag="p", bufs=10,
                                    name=f"p{h}_{st}")
                    nc.scalar.activation(
                        p_sb[:, 0:w], sps[h][:, 0:w],
                        mybir.ActivationFunctionType.Exp, scale=1.0 / 8.0)
                    pbs[h] = p_sb
                for h in heads:
                    p_sb = pbs[h]
                    if k <= 3:
                        # diagonal block: causal zeroing on t' in [128k, w)
                        d0 = 128 * k
                        ceng = nc.vector if tch == 0 else nc.gpsimd
                        ceng.tensor_mul(
                            p_sb[:, d0:w], p_sb[:, d0:w],
                            c01_sb[k][:, 0:w - d0])
                    if m_w > 0:
                        # span mask: pm = min(max(a_h + cneg, 0), 1) * p
                        mt = scr.tile([128, TCH], F16, tag="mt", bufs=6,
                                      name=f"mt{h}_{st}")
                        nc.vector.tensor_scalar(
                            mt[:, 0:m_w], cneg_sb[k][:, 0:m_w],
                            a_sb[:, h:h + 1], 0.0,
                            op0=mybir.AluOpType.add, op1=mybir.AluOpType.max)
                        nc.vector.scalar_tensor_tensor(
                            p_sb[:, 0:m_w], mt[:, 0:m_w], 1.0, p_sb[:, 0:m_w],
                            op0=mybir.AluOpType.min, op1=mybir.AluOpType.mult)
                for h in heads:
                    nc.tensor.matmul(
                        pouts[h][:, 0:w], v_aug[st][:, h, :], pbs[h][:, 0:w],
                        start=(st == first_st), stop=(st == ST - 1),
                        skip_group_check=True)
            for h in heads:
                # rows 0:64 numerator; rows 64:128 denominator W (broadcast)
                et, hp = h // 2, (h % 2) * 64
                pout = pouts[h]
                rw = scr.tile([DH, TCH], F32, tag="rw", bufs=4,
                              name=f"rw{h}")
                with nc.allow_low_precision(reason="denom recip bf16"):
                    nc.vector.reciprocal(rw[:], pout[DH:2 * DH, :])
                nc.vector.scalar_tensor_tensor(
                    out_pair[et][tch][hp:hp + DH, :], pout[0:DH, :], 1.0,
                    rw[:],
                    op0=mybir.AluOpType.mult, op1=mybir.AluOpType.mult)

        def out_proj(tch):
            for tt in range(4 * tch, 4 * (tch + 1)):
                toff = 128 * tt - TCH * tch
                for nch in range(NT):
                    yps = ps_proj.tile([128, TCH], F32, tag="proj")
                    for j in range(4):
                        nc.tensor.matmul(
                            yps[:],
                            out_pair[j][tch][:, toff:toff + 128],
                            wo_sb[j][:, TCH * nch:TCH * (nch + 1)],
                            start=(j == 0), stop=(j == 3))
                    yo = ysb.tile([128, TCH], F32, tag="y")
                    nc.scalar.copy(yo[:], yps[:])
                    nc.sync.dma_start(
                        yp_d[128 * tt:128 * (tt + 1),
                             TCH * nch:TCH * (nch + 1)],
                        yo[:])

        attn_group(0, 0, v_prefetch=True)
        emit_qtkt([2, 3])  # heads 4..7; fills attn bubbles
        attn_group(0, 1)
        out_proj(0)
        attn_group(1, 0)
        attn_group(1, 1)
        out_proj(1)

    nc.compile()
    _NC_CACHE[key] = nc
    return nc


def _prep_core_inputs(x, Wq, Wk, Wv, Wo, Wspan, bspan, cneg, c01):
    bf = ml_dtypes.bfloat16
    in_maps = []
    for c in range(N_CORES):
        b, g = c // 2, c % 2
        hs = slice(E * g, E * (g + 1))
        in_maps.append({
            "c01": c01,
            "xT": np.ascontiguousarray(x[b].T).astype(bf),
            "WqT": np.ascontiguousarray(Wq[hs, :].T).astype(bf),
            "WkT": np.ascontiguousarray(Wk[hs, :].T).astype(bf),
            "WvT": np.ascontiguousarray(Wv[hs, :].T).astype(bf),
            "WoT": np.ascontiguousarray(Wo[:, hs].T).astype(bf),
            "WspT": np.ascontiguousarray(Wspan[HC * g:HC * (g + 1), :].T).astype(bf),
            "bspan": np.asarray(bspan[HC * g:HC * (g + 1)], np.float32).reshape(1, HC),
            "cneg": cneg,
        })
    return in_maps


def _make_c01():
    sp = np.arange(128, dtype=np.float32)[:, None]
    jp = np.arange(128, dtype=np.float32)[None, :]
    return np.stack([(sp - jp >= 0) for _ in range(4)]).astype(np.float16)


def _make_cneg():
    sp = np.arange(128, dtype=np.float32)[:, None]
    tp = np.arange(TCH, dtype=np.float32)[None, :]
    tiles = []
    for k in range(ST):
        d = 128.0 * k + sp - tp
        ramp = -d / R
        ramp = np.where(d < 0, -60000.0, ramp)
        tiles.append(ramp)
    return np.stack(tiles).astype(np.float16)


def kernel(x, Wq, Wk, Wv, Wo, bo, Wspan, bspan):
    x = np.asarray(x, np.float32)
    Wq = np.asarray(Wq, np.float32)
    Wk = np.asarray(Wk, np.float32)
    Wv = np.asarray(Wv, np.float32)
    Wo = np.asarray(Wo, np.float32)
    bo = np.asarray(bo, np.float32)
    Wspan = np.asarray(Wspan, np.float32)
    bspan = np.asarray(bspan, np.float32)

    # span-mask restriction is only exact when every z >= Z_MIN; verify on host
    logits = x.mean(axis=1) @ Wspan.T + bspan
    z = T / (1.0 + np.exp(-logits))
    span_full = bool(z.min() < Z_MIN + 8.0)
    nc = build_nc(span_full=span_full)
    in_maps = _prep_core_inputs(x, Wq, Wk, Wv, Wo, Wspan, bspan, _make_cneg(),
                                _make_c01())
    res = run_bass_kernel_spmd(nc, in_maps, core_ids=list(range(N_CORES)))
    y = np.empty((B, T, D), np.float32)
    for b in range(B):
        y[b] = res.results[2 * b]["yp"] + res.results[2 * b + 1]["yp"] + bo
    return y


# revision 30
# speedup vs baseline: 22527.3672x; 1.0003x over previous
"""AdaptiveSpanAttention Trainium2 kernel (8 NeuronCores).

Sharding: core c -> (batch b = c//2, head-group g = c%2).
Each core computes, for its batch and its 8 heads:
  Q/K/V projections, anti-causal (j>=i) attention with adaptive-span
  mask, renormalization, and a partial output projection
  y_part = Out_g @ Wo[:, e_slice].T  (contraction over its 512 channels).
Host combines: y[b] = y_part[2b] + y_part[2b+1] + bo.

All matmuls in bf16 (f32 PSUM accumulation). Span-mask ramp in fp16
(exact for integer distances). No collectives.
"""
import sys

sys.path.insert(0, "/opt/trn_rl_repo")

from contextlib import ExitStack

import ml_dtypes
import numpy as np

import concourse.bass as bass
import concourse.tile as tile
from concourse import bacc, mybir
from concourse.bass_utils import run_bass_kernel_spmd

BF16 = mybir.dt.bfloat16
F16 = mybir.dt.float16
F32 = mybir.dt.float32

B, T, D, H = 4, 1024, 1024, 16
DH = 64          # head dim
R = 256.0
HC = 8           # heads per core
E = 512          # channels per core (HC * DH)
N_CORES = 8
TCH = 512        # t-chunk width (PSUM f32 free-dim limit)
NT = T // TCH    # 2 t-chunks
ST = T // 128    # 8 s-tiles
DT = D // 128    # 8 d-tiles

_NC_CACHE = {}

Z_MIN = 416.0  # verified on host per-call; span-mask restriction exact when z >= Z_MIN


def causal_width(st, tch):
    """Valid query-column width of block (s_tile=st, t_chunk=tch).

    Block covers s in [128*st, 128*st+128), t in [512*tch, 512*tch+512).
    Valid cells need s >= t, i.e. t' < delta + 128 with
    delta = 128*st - 512*tch.
    """
    delta = 128 * st - 512 * tch
    return max(0, min(TCH, delta + 128))


def span_width(st, tch, span_full):
    """Columns [0, m_w) where the span mask can differ from 1 (given z >= Z_MIN)."""
    delta = 128 * st - 512 * tch
    w = causal_width(st, tch)
    if span_full:
        return w
    return max(0, min(w, delta + 127 - int(Z_MIN)))


def build_nc(span_full=False):
    key = ("nc", span_full)
    if key in _NC_CACHE:
        return _NC_CACHE[key]
    nc = bacc.Bacc("TRN2", target_bir_lowering=False, debug=False, num_devices=1)

    # ---- DRAM parameters (per-core shards prepared on host) ----
    xT_d = nc.declare_dram_parameter("xT", [D, T], BF16, isOutput=False)
    WqT_d = nc.declare_dram_parameter("WqT", [D, E], BF16, isOutput=False)
    WkT_d = nc.declare_dram_parameter("WkT", [D, E], BF16, isOutput=False)
    WvT_d = nc.declare_dram_parameter("WvT", [D, E], BF16, isOutput=False)
    WoT_d = nc.declare_dram_parameter("WoT", [E, D], BF16, isOutput=False)
    WspT_d = nc.declare_dram_parameter("WspT", [D, HC], BF16, isOutput=False)
    bspan_d = nc.declare_dram_parameter("bspan", [1, HC], F32, isOutput=False)
    # cneg[k, s', t'] = -(128k + s' - t')/R, or -60000 where 128k+s'-t' < 0
    cneg_d = nc.declare_dram_parameter("cneg", [ST, 128, TCH], F16, isOutput=False)
    # c01[k, s', j] = 1.0 if s' >= j else 0.0  (causal 0/1 for t' = 128k + j)
    c01_d = nc.declare_dram_parameter("c01", [4, 128, 128], F16, isOutput=False)
    yp_d = nc.declare_dram_parameter("yp", [T, D], F32, isOutput=True)

    with tile.TileContext(nc) as tc, ExitStack() as ctx:
        # ---------------- pools ----------------
        consts = ctx.enter_context(tc.tile_pool(name="consts", bufs=1))
        xp = ctx.enter_context(tc.tile_pool(name="xp", bufs=1))
        wp = ctx.enter_context(tc.tile_pool(name="wp", bufs=1))
        qkp = ctx.enter_context(tc.tile_pool(name="qkp", bufs=1))
        vp = ctx.enter_context(tc.tile_pool(name="vp", bufs=1))
        outp = ctx.enter_context(tc.tile_pool(name="outp", bufs=1))
        scr = ctx.enter_context(tc.tile_pool(name="scr", bufs=3))
        ysb = ctx.enter_context(tc.tile_pool(name="ysb", bufs=3))

        ps_proj = ctx.enter_context(tc.tile_pool(name="ps_proj", bufs=2, space="PSUM"))
        ps_sc = ctx.enter_context(tc.tile_pool(name="ps_sc", bufs=2, space="PSUM"))
        ps_out = ctx.enter_context(tc.tile_pool(name="ps_out", bufs=4, space="PSUM"))

        # ---------------- critical-path loads: xT, Wq, Wk ----------------
        ones_row = consts.tile([1, 128], BF16)
        nc.vector.memset(ones_row[:], 1.0)

        xT_sb, wq_sb, wk_sb = [], [], []
        for dt_i in range(DT):
            t_ = xp.tile([128, T], BF16, tag="xT", bufs=DT, name=f"xT{dt_i}")
            nc.sync.dma_start(t_[:], xT_d[128 * dt_i:128 * (dt_i + 1), :])
            xT_sb.append(t_)
            for lst, dram, tag in ((wq_sb, WqT_d, "wq"), (wk_sb, WkT_d, "wk")):
                t_ = wp.tile([128, E], BF16, tag=tag, bufs=DT,
                             name=f"{tag}{dt_i}")
                nc.sync.dma_start(t_[:], dram[128 * dt_i:128 * (dt_i + 1), :])
                lst.append(t_)

        # ---------------- Q/K projections (transposed layout) ----------------
        # QT[e, t] = sum_d WqT[d, e] * xT[d, t]; psum -> bf16 copies on ACT
        qt_sb = [qkp.tile([128, T], BF16, tag="qt", name=f"qt{i}", bufs=4)
                 for i in range(4)]
        kt_sb = [qkp.tile([128, T], BF16, tag="kt", name=f"kt{i}", bufs=4)
                 for i in range(4)]

        def emit_qtkt(et_list, copy_eng="act"):
            for dst, w_sb in ((qt_sb, wq_sb), (kt_sb, wk_sb)):
                for et in et_list:
                    for tch in range(NT):
                        ps = ps_proj.tile([128, TCH], F32, tag="proj")
                        for dt_i in range(DT):
                            nc.tensor.matmul(
                                ps[:],
                                w_sb[dt_i][:, 128 * et:128 * (et + 1)],
                                xT_sb[dt_i][:, TCH * tch:TCH * (tch + 1)],
                                start=(dt_i == 0), stop=(dt_i == DT - 1))
                        if copy_eng == "act":
                            nc.scalar.copy(
                                dst[et][:, TCH * tch:TCH * (tch + 1)], ps[:])
                        else:
                            nc.vector.tensor_copy(
                                dst[et][:, TCH * tch:TCH * (tch + 1)], ps[:])

        emit_qtkt([0, 1])

        # ---------------- V (natural layout, ones-augmented) ----------------
        # v_aug[st][p, h, 0:64] = V[128*st+p, 64h+j]; v_aug[st][p, h, 64:128] = 1
        # (64 ones columns make attn@V produce the denominator W broadcast
        #  across psum partitions 64:128)
        wv_sb = []
        for dt_i in range(DT):
            t_ = wp.tile([128, E], BF16, tag="wv", bufs=DT, name=f"wv{dt_i}")
            nc.sync.dma_start(t_[:], WvT_d[128 * dt_i:128 * (dt_i + 1), :])
            wv_sb.append(t_)

        v_aug = [None] * ST

        def emit_v(st):
            va = vp.tile([128, HC, 2 * DH], BF16, tag="vaug", bufs=ST,
                         name=f"vaug{st}")
            nc.gpsimd.memset(va[:, :, DH:2 * DH], 1.0)
            ps = ps_proj.tile([128, E], F32, tag="proj")
            for dt_i in range(DT):
                nc.tensor.matmul(
                    ps[:],
                    xT_sb[dt_i][:, 128 * st:128 * (st + 1)],
                    wv_sb[dt_i][:],
                    start=(dt_i == 0), stop=(dt_i == DT - 1))
            nc.vector.tensor_copy(
                va[:, :, 0:DH], ps[:].rearrange("p (h d) -> p h d", h=HC))
            v_aug[st] = va

        emit_v(0)
        emit_v(1)

        # ---------------- remaining loads ----------------
        bspan_sb = consts.tile([1, HC], F32)
        nc.sync.dma_start(bspan_sb[:], bspan_d[:, :])
        wsp_sb = []
        for dt_i in range(DT):
            t_ = wp.tile([128, HC], BF16, tag="wsp", bufs=DT, name=f"wsp{dt_i}")
            nc.sync.dma_start(t_[:], WspT_d[128 * dt_i:128 * (dt_i + 1), :])
            wsp_sb.append(t_)
        wo_sb = []
        for j in range(4):
            t_ = wp.tile([128, D], BF16, tag="wo", bufs=4, name=f"wo{j}")
            nc.sync.dma_start(t_[:], WoT_d[128 * j:128 * (j + 1), :])
            wo_sb.append(t_)
        cneg_sb = []
        for k in range(ST):
            ct = consts.tile([128, TCH], F16, tag="cneg", bufs=ST,
                             name=f"cneg{k}")
            nc.sync.dma_start(ct[:], cneg_d[k])
            cneg_sb.append(ct)
        c01_sb = []
        for k in range(4):
            ct2 = consts.tile([128, 128], F16, tag="c01", bufs=4,
                              name=f"c01_{k}")
            nc.sync.dma_start(ct2[:], c01_d[k])
            c01_sb.append(ct2)

        # ---------------- span net ----------------
        # logit = (sum_t x)/T @ WspanT + bspan; a = 1 + (T/R)*sigmoid(logit)
        msum = consts.tile([128, DT], BF16)
        with nc.allow_low_precision(reason="span-net mean in bf16 is plenty"):
            for dt_i in range(DT):
                nc.vector.tensor_reduce(
                    msum[:, dt_i:dt_i + 1], xT_sb[dt_i][:],
                    mybir.AxisListType.X, mybir.AluOpType.add)
        zlog = ps_sc.tile([1, HC], F32, tag="sc", padded_shape=[128, TCH])
        for dt_i in range(DT):
            nc.tensor.matmul(zlog[:], msum[:, dt_i:dt_i + 1], wsp_sb[dt_i][:],
                             start=(dt_i == 0), stop=(dt_i == DT - 1))
        zrow = consts.tile([1, HC], F32)
        nc.vector.scalar_tensor_tensor(
            zrow[:], zlog[:], 1.0 / T, bspan_sb[:],
            op0=mybir.AluOpType.mult, op1=mybir.AluOpType.add)
        sig = consts.tile([1, HC], BF16)
        nc.scalar.activation(sig[:], zrow[:],
                             mybir.ActivationFunctionType.Sigmoid)
        a_ps = ps_sc.tile([128, HC], F32, tag="sc", padded_shape=[128, TCH])
        nc.tensor.matmul(a_ps[:], ones_row[:], sig[:], start=True, stop=True)
        a_sb = consts.tile([128, HC], F32)
        nc.scalar.activation(a_sb[:], a_ps[:],
                             mybir.ActivationFunctionType.Identity,
                             scale=T / R, bias=1.0)

        # ---------------- attention ----------------
        # out_pair[j][tch] holds heads 2j (parts 0:64) and 2j+1 (parts 64:128)
        out_pair = [[outp.tile([128, TCH], BF16, tag="out", bufs=8,
                               name=f"op{j}_{c}") for c in range(NT)]
                    for j in range(4)]

        def attn_group(tch, hg, v_prefetch=False):
            first_st = 4 * tch
            heads = [4 * hg + i for i in range(4)]
            pouts = {}
            for h in heads:
                pouts[h] = ps_out.tile([128, TCH], F32, tag="pout",
                                       name=f"pout{h}_{tch}")
            for st in range(first_st, ST):
                if v_prefetch and st + 2 < ST and v_aug[st + 2] is None:
                    emit_v(st + 2)
                w = causal_width(st, tch)
                m_w = span_width(st, tch, span_full)
                k = st - first_st  # delta = 128*k
                sps, pbs = {}, {}
                for h in heads:
                    et, hp = h // 2, (h % 2) * 64
                    sp = ps_sc.tile([128, TCH], F32, tag="sc",
                                    name=f"sp{h}_{st}")
                    nc.tensor.matmul(
                        sp[:, 0:w],
                        kt_sb[et][hp:hp + DH, 128 * st:128 * (st + 1)],
                        qt_sb[et][hp:hp + DH, TCH * tch:TCH * tch + w],
                        start=True, stop=True)
                    sps[h] = sp
                for h in heads:
                    p_sb = scr.tile([128, TCH], BF16, tag="p", bufs=10,
                                    name=f"p{h}_{st}")
                    nc.scalar.activation(
                        p_sb[:, 0:w], sps[h][:, 0:w],
                        mybir.ActivationFunctionType.Exp, scale=1.0 / 8.0)
                    pbs[h] = p_sb
                for h in heads:
                    p_sb = pbs[h]
                    if k <= 3:
                        # diagonal block: causal zeroing on t' in [128k, w)
                        d0 = 128 * k
                        ceng = nc.vector if tch == 0 else nc.gpsimd
                        ceng.tensor_mul(
                            p_sb[:, d0:w], p_sb[:, d0:w],
                            c01_sb[k][:, 0:w - d0])
                    if m_w > 0:
                        # span mask: pm = min(max(a_h + cneg, 0), 1) * p
                        mt = scr.tile([128, TCH], F16, tag="mt", bufs=6,
                                      name=f"mt{h}_{st}")
                        nc.vector.tensor_scalar(
                            mt[:, 0:m_w], cneg_sb[k][:, 0:m_w],
                            a_sb[:, h:h + 1], 0.0,
                            op0=mybir.AluOpType.add, op1=mybir.AluOpType.max)
                        nc.vector.scalar_tensor_tensor(
                            p_sb[:, 0:m_w], mt[:, 0:m_w], 1.0, p_sb[:, 0:m_w],
                            op0=mybir.AluOpType.min, op1=mybir.AluOpType.mult)
                for h in heads:
                    nc.tensor.matmul(
                        pouts[h][:, 0:w], v_aug[st][:, h, :], pbs[h][:, 0:w],
                        start=(st == first_st), stop=(st == ST - 1),
                        skip_group_check=True)
            for h in heads:
                # rows 0:64 numerator; rows 64:128 denominator W (broadcast)
                et, hp = h // 2, (h % 2) * 64
                pout = pouts[h]
                rw = scr.tile([DH, TCH], F32, tag="rw", bufs=4,
                              name=f"rw{h}")
                with nc.allow_low_precision(reason="denom recip bf16"):
                    nc.vector.reciprocal(rw[:], pout[DH:2 * DH, :])
                nc.vector.scalar_tensor_tensor(
                    out_pair[et][tch][hp:hp + DH, :], pout[0:DH, :], 1.0,
                    rw[:],
                    op0=mybir.AluOpType.mult, op1=mybir.AluOpType.mult)

        def out_proj(tch):
            for tt in range(4 * tch, 4 * (tch + 1)):
                toff = 128 * tt - TCH * tch
                for nch in range(NT):
                    yps = ps_proj.tile([128, TCH], F32, tag="proj")
                    for j in range(4):
                        nc.tensor.matmul(
                            yps[:],
                            out_pair[j][tch][:, toff:toff + 128],
                            wo_sb[j][:, TCH * nch:TCH * (nch + 1)],
                            start=(j == 0), stop=(j == 3))
                    yo = ysb.tile([128, TCH], F32, tag="y")
                    if tch == 0:
                        nc.vector.tensor_copy(yo[:], yps[:])
                    else:
                        nc.scalar.copy(yo[:], yps[:])
                    nc.sync.dma_start(
                        yp_d[128 * tt:128 * (tt + 1),
                             TCH * nch:TCH * (nch + 1)],
                        yo[:])

        attn_group(0, 0, v_prefetch=True)
        emit_qtkt([2, 3])  # heads 4..7; fills attn bubbles
        attn_group(0, 1)
        out_proj(0)
        attn_group(1, 0)
        attn_group(1, 1)
        out_proj(1)

    nc.compile()
    _NC_CACHE[key] = nc
    return nc


def _prep_core_inputs(x, Wq, Wk, Wv, Wo, Wspan, bspan, cneg, c01):
    bf = ml_dtypes.bfloat16
    in_maps = []
    for c in range(N_CORES):
        b, g = c // 2, c % 2
        hs = slice(E * g, E * (g + 1))
        in_maps.append({
            "c01": c01,
            "xT": np.ascontiguousarray(x[b].T).astype(bf),
            "WqT": np.ascontiguousarray(Wq[hs, :].T).astype(bf),
            "WkT": np.ascontiguousarray(Wk[hs, :].T).astype(bf),
            "WvT": np.ascontiguousarray(Wv[hs, :].T).astype(bf),
            "WoT": np.ascontiguousarray(Wo[:, hs].T).astype(bf),
            "WspT": np.ascontiguousarray(Wspan[HC * g:HC * (g + 1), :].T).astype(bf),
            "bspan": np.asarray(bspan[HC * g:HC * (g + 1)], np.float32).reshape(1, HC),
            "cneg": cneg,
        })
    return in_maps


def _make_c01():
    sp = np.arange(128, dtype=np.float32)[:, None]
    jp = np.arange(128, dtype=np.float32)[None, :]
    return np.stack([(sp - jp >= 0) for _ in range(4)]).astype(np.float16)


def _make_cneg():
    sp = np.arange(128, dtype=np.float32)[:, None]
    tp = np.arange(TCH, dtype=np.float32)[None, :]
    tiles = []
    for k in range(ST):
        d = 128.0 * k + sp - tp
        ramp = -d / R
        ramp = np.where(d < 0, -60000.0, ramp)
        tiles.append(ramp)
    return np.stack(tiles).astype(np.float16)


def kernel(x, Wq, Wk, Wv, Wo, bo, Wspan, bspan):
    x = np.asarray(x, np.float32)
    Wq = np.asarray(Wq, np.float32)
    Wk = np.asarray(Wk, np.float32)
    Wv = np.asarray(Wv, np.float32)
    Wo = np.asarray(Wo, np.float32)
    bo = np.asarray(bo, np.float32)
    Wspan = np.asarray(Wspan, np.float32)
    bspan = np.asarray(bspan, np.float32)

    # span-mask restriction is only exact when every z >= Z_MIN; verify on host
    logits = x.mean(axis=1) @ Wspan.T + bspan
    z = T / (1.0 + np.exp(-logits))
    span_full = bool(z.min() < Z_MIN + 8.0)
    nc = build_nc(span_full=span_full)
    in_maps = _prep_core_inputs(x, Wq, Wk, Wv, Wo, Wspan, bspan, _make_cneg(),
                                _make_c01())
    res = run_bass_kernel_spmd(nc, in_maps, core_ids=list(range(N_CORES)))
    y = np.empty((B, T, D), np.float32)
    for b in range(B):
        y[b] = res.results[2 * b]["yp"] + res.results[2 * b + 1]["yp"] + bo
    return y


# revision 34
# speedup vs baseline: 23386.9881x; 1.0382x over previous
"""AdaptiveSpanAttention Trainium2 kernel (8 NeuronCores).

Sharding: core c -> (batch b = c//2, head-group g = c%2).
Each core computes, for its batch and its 8 heads:
  Q/K/V projections, anti-causal (j>=i) attention with adaptive-span
  mask, renormalization, and a partial output projection
  y_part = Out_g @ Wo[:, e_slice].T  (contraction over its 512 channels).
Host combines: y[b] = y_part[2b] + y_part[2b+1] + bo.

All matmuls in bf16 (f32 PSUM accumulation). Span-mask ramp in fp16
(exact for integer distances). No collectives.
"""
import sys

sys.path.insert(0, "/opt/trn_rl_repo")

from contextlib import ExitStack

import ml_dtypes
import numpy as np

import concourse.bass as bass
import concourse.tile as tile
from concourse import bacc, mybir
from concourse.bass_utils import run_bass_kernel_spmd

BF16 = mybir.dt.bfloat16
F16 = mybir.dt.float16
F32 = mybir.dt.float32

B, T, D, H = 4, 1024, 1024, 16
DH = 64          # head dim
R = 256.0
HC = 8           # heads per core
E = 512          # channels per core (HC * DH)
N_CORES = 8
TCH = 512        # t-chunk width (PSUM f32 free-dim limit)
NT = T // TCH    # 2 t-chunks
ST = T // 128    # 8 s-tiles
DT = D // 128    # 8 d-tiles

_NC_CACHE = {}

Z_MIN = 416.0  # verified on host per-call; span-mask restriction exact when z >= Z_MIN


def causal_width(st, tch):
    """Valid query-column width of block (s_tile=st, t_chunk=tch).

    Block covers s in [128*st, 128*st+128), t in [512*tch, 512*tch+512).
    Valid cells need s >= t, i.e. t' < delta + 128 with
    delta = 128*st - 512*tch.
    """
    delta = 128 * st - 512 * tch
    return max(0, min(TCH, delta + 128))


def span_width(st, tch, span_full):
    """Columns [0, m_w) where the span mask can differ from 1 (given z >= Z_MIN)."""
    delta = 128 * st - 512 * tch
    w = causal_width(st, tch)
    if span_full:
        return w
    return max(0, min(w, delta + 127 - int(Z_MIN)))


def build_nc(span_full=False):
    key = ("nc", span_full)
    if key in _NC_CACHE:
        return _NC_CACHE[key]
    nc = bacc.Bacc("TRN2", target_bir_lowering=False, debug=False, num_devices=1)

    # ---- DRAM parameters (per-core shards prepared on host) ----
    xT_d = nc.declare_dram_parameter("xT", [D, T], BF16, isOutput=False)
    WqT_d = nc.declare_dram_parameter("WqT", [D, E], BF16, isOutput=False)
    WkT_d = nc.declare_dram_parameter("WkT", [D, E], BF16, isOutput=False)
    WvT_d = nc.declare_dram_parameter("WvT", [D, E], BF16, isOutput=False)
    WoT_d = nc.declare_dram_parameter("WoT", [E, D], BF16, isOutput=False)
    WspT_d = nc.declare_dram_parameter("WspT", [D, HC], BF16, isOutput=False)
    bspan_d = nc.declare_dram_parameter("bspan", [1, HC], F32, isOutput=False)
    # packed span-ramp tiles: for each k with nonzero span width m_k, columns
    # [off_k, off_k+m_k) hold cneg[k, s', t'] = -(128k + s' - t')/R
    # (-60000 where causal-invalid)
    widths = [span_width(st, 0, span_full) for st in range(ST)]
    offs = np.concatenate([[0], np.cumsum(widths)]).astype(int)
    cneg_d = nc.declare_dram_parameter("cneg", [128, max(1, int(offs[-1]))],
                                       F16, isOutput=False)
    # c01[k, s', j] = 1.0 if s' >= j else 0.0  (causal 0/1 for t' = 128k + j)
    c01_d = nc.declare_dram_parameter("c01", [4, 128, 128], F16, isOutput=False)
    yp_d = nc.declare_dram_parameter("yp", [T, D], F32, isOutput=True)

    with tile.TileContext(nc) as tc, ExitStack() as ctx:
        # ---------------- pools ----------------
        consts = ctx.enter_context(tc.tile_pool(name="consts", bufs=1))
        xp = ctx.enter_context(tc.tile_pool(name="xp", bufs=1))
        wp = ctx.enter_context(tc.tile_pool(name="wp", bufs=1))
        qkp = ctx.enter_context(tc.tile_pool(name="qkp", bufs=1))
        vp = ctx.enter_context(tc.tile_pool(name="vp", bufs=1))
        outp = ctx.enter_context(tc.tile_pool(name="outp", bufs=1))
        scr = ctx.enter_context(tc.tile_pool(name="scr", bufs=3))
        ysb = ctx.enter_context(tc.tile_pool(name="ysb", bufs=3))

        ps_proj = ctx.enter_context(tc.tile_pool(name="ps_proj", bufs=2, space="PSUM"))
        ps_sc = ctx.enter_context(tc.tile_pool(name="ps_sc", bufs=2, space="PSUM"))
        ps_out = ctx.enter_context(tc.tile_pool(name="ps_out", bufs=2, space="PSUM"))

        # ---------------- critical-path loads: xT, Wq, Wk ----------------
        ones_row = consts.tile([1, 128], BF16)
        nc.vector.memset(ones_row[:], 1.0)

        xT_sb, wq_sb, wk_sb = [], [], []
        for dt_i in range(DT):
            t_ = xp.tile([128, T], BF16, tag="xT", bufs=DT, name=f"xT{dt_i}")
            nc.sync.dma_start(t_[:], xT_d[128 * dt_i:128 * (dt_i + 1), :])
            xT_sb.append(t_)
            for lst, dram, tag in ((wq_sb, WqT_d, "wq"), (wk_sb, WkT_d, "wk")):
                t_ = wp.tile([128, E], BF16, tag=tag, bufs=DT,
                             name=f"{tag}{dt_i}")
                nc.sync.dma_start(t_[:], dram[128 * dt_i:128 * (dt_i + 1), :])
                lst.append(t_)

        # ---------------- Q/K projections (transposed layout) ----------------
        # QT[e, t] = sum_d WqT[d, e] * xT[d, t]; psum -> bf16 copies on ACT
        qt_sb = [qkp.tile([128, T], BF16, tag="qt", name=f"qt{i}", bufs=4)
                 for i in range(4)]
        kt_sb = [qkp.tile([128, T], BF16, tag="kt", name=f"kt{i}", bufs=4)
                 for i in range(4)]

        def emit_qtkt(et_list, copy_eng="act"):
            for dst, w_sb in ((qt_sb, wq_sb), (kt_sb, wk_sb)):
                for et in et_list:
                    for tch in range(NT):
                        ps = ps_proj.tile([128, TCH], F32, tag="proj")
                        for dt_i in range(DT):
                            nc.tensor.matmul(
                                ps[:],
                                w_sb[dt_i][:, 128 * et:128 * (et + 1)],
                                xT_sb[dt_i][:, TCH * tch:TCH * (tch + 1)],
                                start=(dt_i == 0), stop=(dt_i == DT - 1))
                        if copy_eng == "act":
                            nc.scalar.copy(
                                dst[et][:, TCH * tch:TCH * (tch + 1)], ps[:])
                        else:
                            nc.vector.tensor_copy(
                                dst[et][:, TCH * tch:TCH * (tch + 1)], ps[:])

        emit_qtkt([0])

        # ---------------- V (natural layout, ones-augmented) ----------------
        # v_aug[st][p, h, 0:64] = V[128*st+p, 64h+j]; v_aug[st][p, h, 64:128] = 1
        # (64 ones columns make attn@V produce the denominator W broadcast
        #  across psum partitions 64:128)
        wv_sb = []
        for dt_i in range(DT):
            t_ = wp.tile([128, E], BF16, tag="wv", bufs=DT, name=f"wv{dt_i}")
            nc.sync.dma_start(t_[:], WvT_d[128 * dt_i:128 * (dt_i + 1), :])
            wv_sb.append(t_)

        v_aug = [None] * ST

        def emit_v(st):
            va = vp.tile([128, HC, 2 * DH], BF16, tag="vaug", bufs=ST,
                         name=f"vaug{st}")
            nc.gpsimd.memset(va[:, :, DH:2 * DH], 1.0)
            ps = ps_proj.tile([128, E], F32, tag="proj")
            for dt_i in range(DT):
                nc.tensor.matmul(
                    ps[:],
                    xT_sb[dt_i][:, 128 * st:128 * (st + 1)],
                    wv_sb[dt_i][:],
                    start=(dt_i == 0), stop=(dt_i == DT - 1))
            nc.vector.tensor_copy(
                va[:, :, 0:DH], ps[:].rearrange("p (h d) -> p h d", h=HC))
            v_aug[st] = va

        emit_v(0)
        emit_v(1)

        # ---------------- remaining loads ----------------
        bspan_sb = consts.tile([1, HC], F32)
        nc.sync.dma_start(bspan_sb[:], bspan_d[:, :])
        wsp_sb = []
        for dt_i in range(DT):
            t_ = wp.tile([128, HC], BF16, tag="wsp", bufs=DT, name=f"wsp{dt_i}")
            nc.sync.dma_start(t_[:], WspT_d[128 * dt_i:128 * (dt_i + 1), :])
            wsp_sb.append(t_)
        wo_sb = []
        for j in range(4):
            t_ = wp.tile([128, D], BF16, tag="wo", bufs=4, name=f"wo{j}")
            nc.sync.dma_start(t_[:], WoT_d[128 * j:128 * (j + 1), :])
            wo_sb.append(t_)
        cneg_sb = consts.tile([128, max(1, int(offs[-1]))], F16, tag="cneg")
        nc.sync.dma_start(cneg_sb[:], cneg_d[:, :])
        c01_sb = []
        for k in range(4):
            ct2 = consts.tile([128, 128], F16, tag="c01", bufs=4,
                              name=f"c01_{k}")
            nc.sync.dma_start(ct2[:], c01_d[k])
            c01_sb.append(ct2)

        # ---------------- span net ----------------
        # logit = (sum_t x)/T @ WspanT + bspan; a = 1 + (T/R)*sigmoid(logit)
        msum = consts.tile([128, DT], BF16)
        with nc.allow_low_precision(reason="span-net mean in bf16 is plenty"):
            for dt_i in range(DT):
                nc.vector.tensor_reduce(
                    msum[:, dt_i:dt_i + 1], xT_sb[dt_i][:],
                    mybir.AxisListType.X, mybir.AluOpType.add)
        zlog = ps_sc.tile([1, HC], F32, tag="sc", padded_shape=[128, TCH])
        for dt_i in range(DT):
            nc.tensor.matmul(zlog[:], msum[:, dt_i:dt_i + 1], wsp_sb[dt_i][:],
                             start=(dt_i == 0), stop=(dt_i == DT - 1))
        zrow = consts.tile([1, HC], F32)
        nc.vector.scalar_tensor_tensor(
            zrow[:], zlog[:], 1.0 / T, bspan_sb[:],
            op0=mybir.AluOpType.mult, op1=mybir.AluOpType.add)
        sig = consts.tile([1, HC], BF16)
        nc.scalar.activation(sig[:], zrow[:],
                             mybir.ActivationFunctionType.Sigmoid)
        a_ps = ps_sc.tile([128, HC], F32, tag="sc", padded_shape=[128, TCH])
        nc.tensor.matmul(a_ps[:], ones_row[:], sig[:], start=True, stop=True)
        a_sb = consts.tile([128, HC], F32)
        nc.scalar.activation(a_sb[:], a_ps[:],
                             mybir.ActivationFunctionType.Identity,
                             scale=T / R, bias=1.0)

        # ---------------- attention ----------------
        # out_pair[j][tch] holds heads 2j (parts 0:64) and 2j+1 (parts 64:128)
        out_pair = [[outp.tile([128, TCH], BF16, tag="out", bufs=8,
                               name=f"op{j}_{c}") for c in range(NT)]
                    for j in range(4)]

        def attn_pair(tch, j, v_prefetch=False):
            """Attention for head pair (2j, 2j+1); both share et=j.

            Scores for the two heads go into one 2-bank psum pair-tile so a
            single exp covers both; even/odd heads sit at partition bases
            0/64 so their K=64 score matmuls row-pack on silicon.
            """
            first_st = 4 * tch
            heads = (2 * j, 2 * j + 1)
            pouts = [ps_out.tile([128, TCH], F32, tag="pout",
                                 name=f"pout{h}_{tch}") for h in heads]
            for st in range(first_st, ST):
                if v_prefetch and st + 2 < ST and v_aug[st + 2] is None:
                    emit_v(st + 2)
                w = causal_width(st, tch)
                m_w = span_width(st, tch, span_full)
                k = st - first_st  # delta = 128*k
                sc_hp = ps_sc.tile([128, 2, TCH], F32, tag="sc",
                                   name=f"sc{j}_{st}")
                for i, h in enumerate(heads):
                    hp = (h % 2) * 64
                    nc.tensor.matmul(
                        sc_hp[:, i, 0:w],
                        kt_sb[j][hp:hp + DH, 128 * st:128 * (st + 1)],
                        qt_sb[j][hp:hp + DH, TCH * tch:TCH * tch + w],
                        start=True, stop=True)
                p_hp = scr.tile([128, 2, TCH], BF16, tag="p", bufs=6,
                                name=f"p{j}_{st}")
                nc.scalar.activation(
                    p_hp[:, :, 0:w], sc_hp[:, :, 0:w],
                    mybir.ActivationFunctionType.Exp, scale=1.0 / 8.0)
                for i, h in enumerate(heads):
                    if k <= 3:
                        # diagonal block: causal zeroing on t' in [128k, w)
                        d0 = 128 * k
                        ceng = nc.vector if tch == 0 else nc.gpsimd
                        ceng.tensor_mul(
                            p_hp[:, i, d0:w], p_hp[:, i, d0:w],
                            c01_sb[k][:, 0:w - d0])
                    if m_w > 0:
                        # span mask: pm = min(max(a_h + cneg, 0), 1) * p
                        mt = scr.tile([128, TCH], F16, tag="mt", bufs=6,
                                      name=f"mt{h}_{st}")
                        nc.vector.tensor_scalar(
                            mt[:, 0:m_w],
                            cneg_sb[:, offs[k]:offs[k] + m_w],
                            a_sb[:, h:h + 1], 0.0,
                            op0=mybir.AluOpType.add, op1=mybir.AluOpType.max)
                        nc.vector.scalar_tensor_tensor(
                            p_hp[:, i, 0:m_w], mt[:, 0:m_w], 1.0,
                            p_hp[:, i, 0:m_w],
                            op0=mybir.AluOpType.min, op1=mybir.AluOpType.mult)
                for i, h in enumerate(heads):
                    nc.tensor.matmul(
                        pouts[i][:, 0:w], v_aug[st][:, h, :],
                        p_hp[:, i, 0:w],
                        start=(st == first_st), stop=(st == ST - 1),
                        skip_group_check=True)
            for i, h in enumerate(heads):
                # rows 0:64 numerator; rows 64:128 denominator W (broadcast)
                hp = (h % 2) * 64
                pout = pouts[i]
                rw = scr.tile([DH, TCH], F32, tag="rw", bufs=4,
                              name=f"rw{h}")
                with nc.allow_low_precision(reason="denom recip bf16"):
                    nc.vector.reciprocal(rw[:], pout[DH:2 * DH, :])
                nc.vector.scalar_tensor_tensor(
                    out_pair[j][tch][hp:hp + DH, :], pout[0:DH, :], 1.0,
                    rw[:],
                    op0=mybir.AluOpType.mult, op1=mybir.AluOpType.mult)

        def out_proj(tch):
            for tt in range(4 * tch, 4 * (tch + 1)):
                toff = 128 * tt - TCH * tch
                for nch in range(NT):
                    yps = ps_proj.tile([128, TCH], F32, tag="proj")
                    for j in range(4):
                        nc.tensor.matmul(
                            yps[:],
                            out_pair[j][tch][:, toff:toff + 128],
                            wo_sb[j][:, TCH * nch:TCH * (nch + 1)],
                            start=(j == 0), stop=(j == 3))
                    yo = ysb.tile([128, TCH], F32, tag="y")
                    if tch == 0:
                        nc.vector.tensor_copy(yo[:], yps[:])
                    else:
                        nc.scalar.copy(yo[:], yps[:])
                    nc.sync.dma_start(
                        yp_d[128 * tt:128 * (tt + 1),
                             TCH * nch:TCH * (nch + 1)],
                        yo[:])

        attn_pair(0, 0, v_prefetch=True)
        emit_qtkt([1])
        attn_pair(0, 1)
        emit_qtkt([2])
        attn_pair(0, 2)
        emit_qtkt([3])
        attn_pair(0, 3)
        out_proj(0)
        for j in range(4):
            attn_pair(1, j)
        out_proj(1)

    nc.compile()
    _NC_CACHE[key] = nc
    return nc


def _prep_core_inputs(x, Wq, Wk, Wv, Wo, Wspan, bspan, cneg, c01):
    bf = ml_dtypes.bfloat16
    in_maps = []
    for c in range(N_CORES):
        b, g = c // 2, c % 2
        hs = slice(E * g, E * (g + 1))
        in_maps.append({
            "c01": c01,
            "xT": np.ascontiguousarray(x[b].T).astype(bf),
            "WqT": np.ascontiguousarray(Wq[hs, :].T).astype(bf),
            "WkT": np.ascontiguousarray(Wk[hs, :].T).astype(bf),
            "WvT": np.ascontiguousarray(Wv[hs, :].T).astype(bf),
            "WoT": np.ascontiguousarray(Wo[:, hs].T).astype(bf),
            "WspT": np.ascontiguousarray(Wspan[HC * g:HC * (g + 1), :].T).astype(bf),
            "bspan": np.asarray(bspan[HC * g:HC * (g + 1)], np.float32).reshape(1, HC),
            "cneg": cneg,
        })
    return in_maps


def _make_c01():
    sp = np.arange(128, dtype=np.float32)[:, None]
    jp = np.arange(128, dtype=np.float32)[None, :]
    return np.stack([(sp - jp >= 0) for _ in range(4)]).astype(np.float16)


def _make_cneg(span_full):
    sp = np.arange(128, dtype=np.float32)[:, None]
    cols = []
    for k in range(ST):
        m_w = span_width(k, 0, span_full)
        if m_w == 0:
            continue
        tp = np.arange(m_w, dtype=np.float32)[None, :]
        d = 128.0 * k + sp - tp
        ramp = np.where(d < 0, -60000.0, -d / R)
        cols.append(ramp)
    if not cols:
        return np.zeros((128, 1), np.float16)
    return np.concatenate(cols, axis=1).astype(np.float16)


def kernel(x, Wq, Wk, Wv, Wo, bo, Wspan, bspan):
    x = np.asarray(x, np.float32)
    Wq = np.asarray(Wq, np.float32)
    Wk = np.asarray(Wk, np.float32)
    Wv = np.asarray(Wv, np.float32)
    Wo = np.asarray(Wo, np.float32)
    bo = np.asarray(bo, np.float32)
    Wspan = np.asarray(Wspan, np.float32)
    bspan = np.asarray(bspan, np.float32)

    # span-mask restriction is only exact when every z >= Z_MIN; verify on host
    logits = x.mean(axis=1) @ Wspan.T + bspan
    z = T / (1.0 + np.exp(-logits))
    span_full = bool(z.min() < Z_MIN + 8.0)
    nc = build_nc(span_full=span_full)
    in_maps = _prep_core_inputs(x, Wq, Wk, Wv, Wo, Wspan, bspan,
                                _make_cneg(span_full), _make_c01())
    res = run_bass_kernel_spmd(nc, in_maps, core_ids=list(range(N_CORES)))
    y = np.empty((B, T, D), np.float32)
    for b in range(B):
        y[b] = res.results[2 * b]["yp"] + res.results[2 * b + 1]["yp"] + bo
    return y


# revision 46
# speedup vs baseline: 24438.2491x; 1.0450x over previous
"""AdaptiveSpanAttention Trainium2 kernel (8 NeuronCores).

Sharding: core c -> (batch b = c//2, head-group g = c%2).
Each core computes, for its batch and its 8 heads:
  Q/K/V projections, anti-causal (j>=i) attention with adaptive-span
  mask, renormalization, and a partial output projection
  y_part = Out_g @ Wo[:, e_slice].T  (contraction over its 512 channels).
Host combines: y[b] = y_part[2b] + y_part[2b+1] + bo.

All matmuls in bf16 (f32 PSUM accumulation). Span-mask ramp in fp16
(exact for integer distances). No collectives.
"""
import sys

sys.path.insert(0, "/opt/trn_rl_repo")

from contextlib import ExitStack

import ml_dtypes
import numpy as np

import concourse.bass as bass
import concourse.tile as tile
from concourse import bacc, mybir
from concourse.bass_utils import run_bass_kernel_spmd

BF16 = mybir.dt.bfloat16
F16 = mybir.dt.float16
F32 = mybir.dt.float32

B, T, D, H = 4, 1024, 1024, 16
DH = 64          # head dim
R = 256.0
HC = 8           # heads per core
E = 512          # channels per core (HC * DH)
N_CORES = 8
TCH = 512        # t-chunk width (PSUM f32 free-dim limit)
NT = T // TCH    # 2 t-chunks
ST = T // 128    # 8 s-tiles
DT = D // 128    # 8 d-tiles

_NC_CACHE = {}

Z_MIN = 416.0  # verified on host per-call; span-mask restriction exact when z >= Z_MIN


def causal_width(st, tch):
    """Valid query-column width of block (s_tile=st, t_chunk=tch).

    Block covers s in [128*st, 128*st+128), t in [512*tch, 512*tch+512).
    Valid cells need s >= t, i.e. t' < delta + 128 with
    delta = 128*st - 512*tch.
    """
    delta = 128 * st - 512 * tch
    return max(0, min(TCH, delta + 128))


def span_width(st, tch, span_full):
    """Columns [0, m_w) where the span mask can differ from 1 (given z >= Z_MIN)."""
    delta = 128 * st - 512 * tch
    w = causal_width(st, tch)
    if span_full:
        return w
    return max(0, min(w, delta + 127 - int(Z_MIN)))


def build_nc(span_full=False):
    key = ("nc", span_full)
    if key in _NC_CACHE:
        return _NC_CACHE[key]
    nc = bacc.Bacc("TRN2", target_bir_lowering=False, debug=False, num_devices=1)

    # ---- DRAM parameters (per-core shards prepared on host) ----
    xT_d = nc.declare_dram_parameter("xT", [D, T], BF16, isOutput=False)
    WqT_d = nc.declare_dram_parameter("WqT", [D, E], BF16, isOutput=False)
    WkT_d = nc.declare_dram_parameter("WkT", [D, E], BF16, isOutput=False)
    WvT_d = nc.declare_dram_parameter("WvT", [D, E], BF16, isOutput=False)
    WoT_d = nc.declare_dram_parameter("WoT", [E, D], BF16, isOutput=False)
    WspT_d = nc.declare_dram_parameter("WspT", [D, HC], BF16, isOutput=False)
    bspan_d = nc.declare_dram_parameter("bspan", [1, HC], F32, isOutput=False)
    # packed span-ramp tiles: for each k with nonzero span width m_k, columns
    # [off_k, off_k+m_k) hold cneg[k, s', t'] = -(128k + s' - t')/R
    # (-60000 where causal-invalid)
    widths = [span_width(st, 0, span_full) for st in range(ST)]
    offs = np.concatenate([[0], np.cumsum(widths)]).astype(int)
    cneg_d = nc.declare_dram_parameter("cneg", [128, max(1, int(offs[-1]))],
                                       F16, isOutput=False)
    # c01[k, s', j] = 1.0 if s' >= j else 0.0  (causal 0/1 for t' = 128k + j)
    c01_d = nc.declare_dram_parameter("c01", [4, 128, 128], F16, isOutput=False)
    yp_d = nc.declare_dram_parameter("yp", [T, D], F32, isOutput=True)

    with tile.TileContext(nc) as tc, ExitStack() as ctx:
        # ---------------- pools ----------------
        consts = ctx.enter_context(tc.tile_pool(name="consts", bufs=1))
        xp = ctx.enter_context(tc.tile_pool(name="xp", bufs=1))
        wp = ctx.enter_context(tc.tile_pool(name="wp", bufs=1))
        qkp = ctx.enter_context(tc.tile_pool(name="qkp", bufs=1))
        vp = ctx.enter_context(tc.tile_pool(name="vp", bufs=1))
        outp = ctx.enter_context(tc.tile_pool(name="outp", bufs=1))
        scr = ctx.enter_context(tc.tile_pool(name="scr", bufs=3))
        ysb = ctx.enter_context(tc.tile_pool(name="ysb", bufs=3))

        ps_proj = ctx.enter_context(tc.tile_pool(name="ps_proj", bufs=2, space="PSUM"))
        lead_ctx = ExitStack()
        ps_lead = lead_ctx.enter_context(
            tc.tile_pool(name="ps_lead", bufs=6, space="PSUM"))

        # ---------------- critical-path loads: xT, Wq, Wk ----------------
        ones_row = consts.tile([1, 128], BF16)
        nc.vector.memset(ones_row[:], 1.0)

        xT_sb, wq_sb, wk_sb = [], [], []
        for dt_i in range(DT):
            t_ = xp.tile([128, T], BF16, tag="xT", bufs=DT, name=f"xT{dt_i}")
            nc.sync.dma_start(t_[:], xT_d[128 * dt_i:128 * (dt_i + 1), :])
            xT_sb.append(t_)
            t_ = wp.tile([128, E], BF16, tag="wq", bufs=DT, name=f"wq{dt_i}")
            nc.sync.dma_start(t_[:], WqT_d[128 * dt_i:128 * (dt_i + 1), :])
            wq_sb.append(t_)
        for dt_i in range(DT):
            t_ = wp.tile([128, E], BF16, tag="wk", bufs=DT, name=f"wk{dt_i}")
            nc.sync.dma_start(t_[:], WkT_d[128 * dt_i:128 * (dt_i + 1), :])
            wk_sb.append(t_)

        # span-net partial sums early: each reduce runs as its xT tile lands,
        # hidden under the DMA lead-in
        msum = consts.tile([128, DT], BF16)
        with nc.allow_low_precision(reason="span-net mean in bf16 is plenty"):
            for dt_i in range(DT):
                nc.vector.tensor_reduce(
                    msum[:, dt_i:dt_i + 1], xT_sb[dt_i][:],
                    mybir.AxisListType.X, mybir.AluOpType.add)

        # ---------------- Q/K projections (transposed layout) ----------------
        # QT[e, t] = sum_d WqT[d, e] * xT[d, t]; psum -> bf16 copies on ACT
        qt_sb = [qkp.tile([128, T], BF16, tag="qt", name=f"qt{i}", bufs=4)
                 for i in range(4)]
        kt_sb = [qkp.tile([128, T], BF16, tag="kt", name=f"kt{i}", bufs=4)
                 for i in range(4)]

        grp_ctr = [0]

        def emit_qtkt(et_list, copy_eng="act", pool=None):
            pool = pool or ps_proj
            for dst, w_sb in ((qt_sb, wq_sb), (kt_sb, wk_sb)):
                for et in et_list:
                    for tch in range(NT):
                        ps = pool.tile([128, TCH], F32, tag="pj",
                                       name=f"pj{et}_{tch}")
                        # rotate the contraction start so concurrent groups
                        # finish at different DMA-front positions
                        rot = 0
                        grp_ctr[0] += 1
                        order = [(rot + i) % DT for i in range(DT)]
                        for n_i, dt_i in enumerate(order):
                            nc.tensor.matmul(
                                ps[:],
                                w_sb[dt_i][:, 128 * et:128 * (et + 1)],
                                xT_sb[dt_i][:, TCH * tch:TCH * (tch + 1)],
                                start=(n_i == 0), stop=(n_i == DT - 1))
                        if copy_eng == "act":
                            nc.scalar.copy(
                                dst[et][:, TCH * tch:TCH * (tch + 1)], ps[:])
                        else:
                            nc.vector.tensor_copy(
                                dst[et][:, TCH * tch:TCH * (tch + 1)], ps[:])

        emit_qtkt([0], pool=ps_lead)

        # ---------------- V (natural layout, ones-augmented) ----------------
        # v_aug[st][p, h, 0:64] = V[128*st+p, 64h+j]; v_aug[st][p, h, 64:128] = 1
        # (64 ones columns make attn@V produce the denominator W broadcast
        #  across psum partitions 64:128)
        wv_sb = []
        for dt_i in range(DT):
            t_ = wp.tile([128, E], BF16, tag="wv", bufs=DT, name=f"wv{dt_i}")
            nc.sync.dma_start(t_[:], WvT_d[128 * dt_i:128 * (dt_i + 1), :])
            wv_sb.append(t_)

        v_aug = [None] * ST

        def emit_v(st, pool=None):
            pool = pool or ps_proj
            va = vp.tile([128, HC, 2 * DH], BF16, tag="vaug", bufs=ST,
                         name=f"vaug{st}")
            nc.gpsimd.memset(va[:, :, DH:2 * DH], 1.0)
            ps = pool.tile([128, E], F32, tag="pj", name=f"pjv{st}")
            rot = 0
            grp_ctr[0] += 1
            order = [(rot + i) % DT for i in range(DT)]
            for n_i, dt_i in enumerate(order):
                nc.tensor.matmul(
                    ps[:],
                    xT_sb[dt_i][:, 128 * st:128 * (st + 1)],
                    wv_sb[dt_i][:],
                    start=(n_i == 0), stop=(n_i == DT - 1))
            nc.vector.tensor_copy(
                va[:, :, 0:DH], ps[:].rearrange("p (h d) -> p h d", h=HC))
            v_aug[st] = va

        emit_v(0, pool=ps_lead)
        emit_v(1, pool=ps_lead)
        emit_qtkt([1], pool=ps_lead)

        # ---------------- remaining loads ----------------
        bspan_sb = consts.tile([1, HC], F32)
        nc.sync.dma_start(bspan_sb[:], bspan_d[:, :])
        wsp_sb = []
        for dt_i in range(DT):
            t_ = wp.tile([128, HC], BF16, tag="wsp", bufs=DT, name=f"wsp{dt_i}")
            nc.sync.dma_start(t_[:], WspT_d[128 * dt_i:128 * (dt_i + 1), :])
            wsp_sb.append(t_)
        wo_sb = []
        for j in range(4):
            t_ = wp.tile([128, D], BF16, tag="wo", bufs=4, name=f"wo{j}")
            nc.sync.dma_start(t_[:], WoT_d[128 * j:128 * (j + 1), :])
            wo_sb.append(t_)
        cneg_sb = consts.tile([128, max(1, int(offs[-1]))], F16, tag="cneg")
        nc.sync.dma_start(cneg_sb[:], cneg_d[:, :])
        c01_sb = []
        for k in range(4):
            ct2 = consts.tile([128, 128], F16, tag="c01", bufs=4,
                              name=f"c01_{k}")
            nc.sync.dma_start(ct2[:], c01_d[k])
            c01_sb.append(ct2)

        # ---------------- span net ----------------
        # logit = (sum_t x)/T @ WspanT + bspan; a = 1 + (T/R)*sigmoid(logit)
        zlog = ps_lead.tile([1, HC], F32, tag="pj", padded_shape=[128, TCH])
        for dt_i in range(DT):
            nc.tensor.matmul(zlog[:], msum[:, dt_i:dt_i + 1], wsp_sb[dt_i][:],
                             start=(dt_i == 0), stop=(dt_i == DT - 1))
        zrow = consts.tile([1, HC], F32)
        nc.vector.scalar_tensor_tensor(
            zrow[:], zlog[:], 1.0 / T, bspan_sb[:],
            op0=mybir.AluOpType.mult, op1=mybir.AluOpType.add)
        sig = consts.tile([1, HC], BF16)
        nc.scalar.activation(sig[:], zrow[:],
                             mybir.ActivationFunctionType.Sigmoid)
        a_ps = ps_lead.tile([128, HC], F32, tag="pj", padded_shape=[128, TCH])
        nc.tensor.matmul(a_ps[:], ones_row[:], sig[:], start=True, stop=True)
        a_sb = consts.tile([128, HC], F32)
        nc.scalar.activation(a_sb[:], a_ps[:],
                             mybir.ActivationFunctionType.Identity,
                             scale=T / R, bias=1.0)

        lead_ctx.close()
        ps_sc = ctx.enter_context(tc.tile_pool(name="ps_sc", bufs=2, space="PSUM"))
        ps_out = ctx.enter_context(tc.tile_pool(name="ps_out", bufs=2, space="PSUM"))

        # ---------------- attention ----------------
        # out_pair[j][tch] holds heads 2j (parts 0:64) and 2j+1 (parts 64:128)
        out_pair = [[outp.tile([128, TCH], BF16, tag="out", bufs=8,
                               name=f"op{j}_{c}") for c in range(NT)]
                    for j in range(4)]

        def attn_pair(tch, j, v_prefetch=False):
            """Attention for head pair (2j, 2j+1); both share et=j.

            Scores for the two heads go into one 2-bank psum pair-tile so a
            single exp covers both; even/odd heads sit at partition bases
            0/64 so their K=64 score matmuls row-pack on silicon.
            """
            first_st = 4 * tch
            heads = (2 * j, 2 * j + 1)
            pouts = [ps_out.tile([128, TCH], F32, tag="pout",
                                 name=f"pout{h}_{tch}") for h in heads]
            for st in range(first_st, ST):
                if v_prefetch and st + 2 < ST and v_aug[st + 2] is None:
                    emit_v(st + 2)
                w = causal_width(st, tch)
                m_w = span_width(st, tch, span_full)
                k = st - first_st  # delta = 128*k
                sc_hp = ps_sc.tile([128, 2, TCH], F32, tag="sc",
                                   name=f"sc{j}_{st}")
                for i, h in enumerate(heads):
                    hp = (h % 2) * 64
                    nc.tensor.matmul(
                        sc_hp[:, i, 0:w],
                        kt_sb[j][hp:hp + DH, 128 * st:128 * (st + 1)],
                        qt_sb[j][hp:hp + DH, TCH * tch:TCH * tch + w],
                        start=True, stop=True)
                p_hp = scr.tile([128, 2, TCH], BF16, tag="p", bufs=6,
                                name=f"p{j}_{st}")
                nc.scalar.activation(
                    p_hp[:, :, 0:w], sc_hp[:, :, 0:w],
                    mybir.ActivationFunctionType.Exp, scale=1.0 / 8.0)
                for i, h in enumerate(heads):
                    if k <= 3:
                        # diagonal block: causal zeroing on t' in [128k, w)
                        d0 = 128 * k
                        ceng = nc.vector if tch == 0 else nc.gpsimd
                        ceng.tensor_mul(
                            p_hp[:, i, d0:w], p_hp[:, i, d0:w],
                            c01_sb[k][:, 0:w - d0])
                    if m_w > 0:
                        # span mask: pm = min(max(a_h + cneg, 0), 1) * p
                        mt = scr.tile([128, TCH], F16, tag="mt", bufs=6,
                                      name=f"mt{h}_{st}")
                        nc.vector.tensor_scalar(
                            mt[:, 0:m_w],
                            cneg_sb[:, offs[k]:offs[k] + m_w],
                            a_sb[:, h:h + 1], 0.0,
                            op0=mybir.AluOpType.add, op1=mybir.AluOpType.max)
                        nc.vector.scalar_tensor_tensor(
                            p_hp[:, i, 0:m_w], mt[:, 0:m_w], 1.0,
                            p_hp[:, i, 0:m_w],
                            op0=mybir.AluOpType.min, op1=mybir.AluOpType.mult)
                for i, h in enumerate(heads):
                    nc.tensor.matmul(
                        pouts[i][:, 0:w], v_aug[st][:, h, :],
                        p_hp[:, i, 0:w],
                        start=(st == first_st), stop=(st == ST - 1),
                        skip_group_check=True)
            for i, h in enumerate(heads):
                # rows 0:64 numerator; rows 64:128 denominator W (broadcast)
                hp = (h % 2) * 64
                pout = pouts[i]
                rw = scr.tile([DH, TCH], F32, tag="rw", bufs=4,
                              name=f"rw{h}")
                with nc.allow_low_precision(reason="denom recip bf16"):
                    nc.vector.reciprocal(rw[:], pout[DH:2 * DH, :])
                nc.vector.scalar_tensor_tensor(
                    out_pair[j][tch][hp:hp + DH, :], pout[0:DH, :], 1.0,
                    rw[:],
                    op0=mybir.AluOpType.mult, op1=mybir.AluOpType.mult)

        def out_proj(tch):
            for tt in range(4 * tch, 4 * (tch + 1)):
                toff = 128 * tt - TCH * tch
                for nch in range(NT):
                    yps = ps_proj.tile([128, TCH], F32, tag="pj",
                                       name=f"y{tt}_{nch}")
                    for j in range(4):
                        nc.tensor.matmul(
                            yps[:],
                            out_pair[j][tch][:, toff:toff + 128],
                            wo_sb[j][:, TCH * nch:TCH * (nch + 1)],
                            start=(j == 0), stop=(j == 3))
                    yo = ysb.tile([128, TCH], F32, tag="y")
                    if tch == 0:
                        nc.vector.tensor_copy(yo[:], yps[:])
                    else:
                        nc.scalar.copy(yo[:], yps[:])
                    nc.sync.dma_start(
                        yp_d[128 * tt:128 * (tt + 1),
                             TCH * nch:TCH * (nch + 1)],
                        yo[:])

        attn_pair(0, 0, v_prefetch=True)
        attn_pair(0, 1)
        emit_qtkt([2])
        attn_pair(0, 2)
        emit_qtkt([3])
        attn_pair(0, 3)
        out_proj(0)
        for j in range(4):
            attn_pair(1, j)
        out_proj(1)

    nc.compile()
    _NC_CACHE[key] = nc
    return nc


def _prep_core_inputs(x, Wq, Wk, Wv, Wo, Wspan, bspan, cneg, c01):
    bf = ml_dtypes.bfloat16
    in_maps = []
    for c in range(N_CORES):
        b, g = c // 2, c % 2
        hs = slice(E * g, E * (g + 1))
        in_maps.append({
            "c01": c01,
            "xT": np.ascontiguousarray(x[b].T).astype(bf),
            "WqT": np.ascontiguousarray(Wq[hs, :].T).astype(bf),
            "WkT": np.ascontiguousarray(Wk[hs, :].T).astype(bf),
            "WvT": np.ascontiguousarray(Wv[hs, :].T).astype(bf),
            "WoT": np.ascontiguousarray(Wo[:, hs].T).astype(bf),
            "WspT": np.ascontiguousarray(Wspan[HC * g:HC * (g + 1), :].T).astype(bf),
            "bspan": np.asarray(bspan[HC * g:HC * (g + 1)], np.float32).reshape(1, HC),
            "cneg": cneg,
        })
    return in_maps


def _make_c01():
    sp = np.arange(128, dtype=np.float32)[:, None]
    jp = np.arange(128, dtype=np.float32)[None, :]
    return np.stack([(sp - jp >= 0) for _ in range(4)]).astype(np.float16)


def _make_cneg(span_full):
    sp = np.arange(128, dtype=np.float32)[:, None]
    cols = []
    for k in range(ST):
        m_w = span_width(k, 0, span_full)
        if m_w == 0:
            continue
        tp = np.arange(m_w, dtype=np.float32)[None, :]
        d = 128.0 * k + sp - tp
        ramp = np.where(d < 0, -60000.0, -d / R)
        cols.append(ramp)
    if not cols:
        return np.zeros((128, 1), np.float16)
    return np.concatenate(cols, axis=1).astype(np.float16)


def kernel(x, Wq, Wk, Wv, Wo, bo, Wspan, bspan):
    x = np.asarray(x, np.float32)
    Wq = np.asarray(Wq, np.float32)
    Wk = np.asarray(Wk, np.float32)
    Wv = np.asarray(Wv, np.float32)
    Wo = np.asarray(Wo, np.float32)
    bo = np.asarray(bo, np.float32)
    Wspan = np.asarray(Wspan, np.float32)
    bspan = np.asarray(bspan, np.float32)

    # span-mask restriction is only exact when every z >= Z_MIN; verify on host
    logits = x.mean(axis=1) @ Wspan.T + bspan
    z = T / (1.0 + np.exp(-logits))
    span_full = bool(z.min() < Z_MIN + 8.0)
    nc = build_nc(span_full=span_full)
    in_maps = _prep_core_inputs(x, Wq, Wk, Wv, Wo, Wspan, bspan,
                                _make_cneg(span_full), _make_c01())
    res = run_bass_kernel_spmd(nc, in_maps, core_ids=list(range(N_CORES)))
    y = np.empty((B, T, D), np.float32)
    for b in range(B):
        y[b] = res.results[2 * b]["yp"] + res.results[2 * b + 1]["yp"] + bo
    return y
